# revision 17
# baseline (speedup 1.0000x reference)
"""MultiHeadLatentAttention TRN2 kernel (v2, bf16).

Sharding: 8 cores = 2 (batch) x 4 (head groups of 4 heads).
Each core computes, for its batch b and heads hg*4..hg*4+3:
  - latent down-projections kv_d, q_d (replicated within the batch group)
  - per-head up-projections K^T, Q^T (with RoPE), V
  - full attention for its 4 heads
  - partial output projection (its 512 columns of Wo's input dim)
Partial outputs are summed on the host (+ bo).

Optimizations vs the first working version (697us/core -> 463us/core on
HW, NTFF-profiled):
  - all matmul operands in bf16 (tolerance test: full-pipeline max-rel
    error ~3e-3 vs the 2e-2 gate); halves LDWEIGHTS + SBUF + DMA.
  - all weights + cos/sin resident in SBUF, loaded once (v1 reloaded
    down-proj weights every s-chunk: ~30MB of redundant DMA); x chunks
    double-buffered and prefetched ahead of the weight preloads so the
    first matmul starts early.
  - softmax row-sum accumulation as two interleaved bf16 chains on DVE
    (all-16-bit DVE ops run at 2x; the Pool engine measured ~2.5ns/elem
    and can't read PSUM, so it only does the 1/sum partition-broadcast);
    1/sum via reciprocal_approx_fast; PSUM double-buffered so the
    softmax epilogue never blocks the next head's QK matmuls.
  - output written in bf16 (halves the HBM write traffic that stalled
    the phase-C PSUM drain).
Phase floors per core: A (projections) ~175us PE-bound, B (attention)
~190us ACT-bound (16.8M exp elements at 0.833ns/elem), C (out-proj)
~65us PE-bound.
"""

import sys

sys.path.insert(0, "/opt/trn_rl_repo")

from contextlib import ExitStack

import numpy as np

H = 16
E = 2048
LAT = E // 4          # 512
D = E // H            # 128
R = D // 2            # 64
B, S = 2, 2048
HPC = H // 4          # 4 heads per core
NCORES = 8
NE = E // 128         # 16 contraction chunks over E
NL = LAT // 128       # 4 contraction chunks over LAT
SW = 512              # s-chunk width for projections
NSC = S // SW         # 4 s-chunks
NKC = S // 128        # 16 key chunks
SCALE = 1.0 / float(np.sqrt(D))

_RT = {}  # cached runtimes


def _mk(nc):
    """Declare DRAM I/O; returns dict of handles."""
    import concourse.mybir as mybir
    F32 = mybir.dt.float32
    F32R = mybir.dt.float32r
    BF16 = mybir.dt.bfloat16
    d = {}
    d["xT"] = nc.dram_tensor("xT", [E, S], BF16, kind="ExternalInput")
    d["wkvdT"] = nc.dram_tensor("wkvdT", [E, LAT], BF16, kind="ExternalInput")
    d["wqdT"] = nc.dram_tensor("wqdT", [E, LAT], BF16, kind="ExternalInput")
    d["wrkT"] = nc.dram_tensor("wrkT", [E, HPC * R], BF16,
                               kind="ExternalInput")
    d["wkuT"] = nc.dram_tensor("wkuT", [LAT, HPC * R], BF16,
                               kind="ExternalInput")
    d["wquT"] = nc.dram_tensor("wquT", [LAT, HPC * R], BF16,
                               kind="ExternalInput")
    d["wrqT"] = nc.dram_tensor("wrqT", [LAT, HPC * R], BF16,
                               kind="ExternalInput")
    d["wvuT"] = nc.dram_tensor("wvuT", [LAT, HPC * D], BF16,
                               kind="ExternalInput")
    d["woT"] = nc.dram_tensor("woT", [HPC * D, E], BF16,
                              kind="ExternalInput")
    d["bkvd"] = nc.dram_tensor("bkvd", [128, NL], F32, kind="ExternalInput")
    d["bqd"] = nc.dram_tensor("bqd", [128, NL], F32, kind="ExternalInput")
    d["bku"] = nc.dram_tensor("bku", [128, 2], F32, kind="ExternalInput")
    d["bqu"] = nc.dram_tensor("bqu", [128, 2], F32, kind="ExternalInput")
    d["brk"] = nc.dram_tensor("brk", [128, 2], F32, kind="ExternalInput")
    d["brq"] = nc.dram_tensor("brq", [128, 2], F32, kind="ExternalInput")
    d["bvu"] = nc.dram_tensor("bvu", [1, HPC * D], F32, kind="ExternalInput")
    d["onesd"] = nc.dram_tensor("onesd", [128, 1], F32R,
                                kind="ExternalInput")
    d["cosT"] = nc.dram_tensor("cosT", [128, S], BF16, kind="ExternalInput")
    d["sinT"] = nc.dram_tensor("sinT", [128, S], BF16, kind="ExternalInput")
    d["out"] = nc.dram_tensor("out", [S, E], BF16, kind="ExternalOutput")
    return d


def _consts(nc, tc, top, d, loads=True):
    """Persistent tiles: K/Q/V storage, biases, ones, all weights.
    With loads=False only allocates; call _consts_load to emit DMAs."""
    import concourse.mybir as mybir
    F32 = mybir.dt.float32
    F32R = mybir.dt.float32r
    BF16 = mybir.dt.bfloat16

    kq_pool = top.enter_context(tc.tile_pool(name="kq", bufs=1))
    v_pool = top.enter_context(tc.tile_pool(name="vp", bufs=1))
    cpool = top.enter_context(tc.tile_pool(name="cp", bufs=1))

    t = {}
    t["K"] = [kq_pool.tile([128, S], BF16, name=f"Kt{h}") for h in range(HPC)]
    t["Q"] = [kq_pool.tile([128, S], BF16, name=f"Qt{h}") for h in range(HPC)]
    t["V"] = [v_pool.tile([128, HPC * D], BF16, name=f"Vt{i}")
              for i in range(NKC)]

    def al(name, shape, dt=F32):
        return cpool.tile(shape, dt, name=name)

    # Down-projection weights: per output m-chunk, a [128, NE*128] tile
    # whose e-th column block is the lhsT for contraction chunk e.
    t["dnw"] = {
        nm: [al(f"wd_{nm}{m}", [128, NE * 128], BF16)
             for m in range(nm_chunks)]
        for nm, nm_chunks in (("kvd", NL), ("qd", NL), ("rk", 2))
    }
    t["upw"] = {
        nm: [al(f"w{nm}{l}", [128, w], BF16) for l in range(NL)]
        for nm, w in (("ku", HPC * R), ("qu", HPC * R),
                      ("rq", HPC * R), ("vu", HPC * D))
    }
    t["ones"] = al("ones_t", [128, 1], F32R)
    t["bkvd"] = al("bkvd_t", [128, NL])
    t["bqd"] = al("bqd_t", [128, NL])
    t["bku"] = al("bku_t", [128, 2])
    t["bqu"] = al("bqu_t", [128, 2])
    t["brk"] = al("brk_t", [128, 2])
    t["brq"] = al("brq_t", [128, 2])
    t["cos"] = al("cos_t", [128, S], BF16)
    t["sin"] = al("sin_t", [128, S], BF16)
    t["bvu_row"] = al("bvu_row", [1, HPC * D])
    t["bvu_bc"] = al("bvu_bc", [128, HPC * D])
    # wo loads are issued later (before phase C) via _load_wo
    t["wo"] = [al(f"wo{hc}", [128, E], BF16) for hc in range(HPC)]
    if loads:
        _consts_load(nc, t, d)
    return t


def _consts_load(nc, t, d):
    """Emit the persistent-tile DMAs: kv_d weights first (phase A's
    first matmul needs only those + x chunk 0)."""
    dmap = {"kvd": "wkvdT", "qd": "wqdT", "rk": "wrkT"}
    for nm in ("kvd", "qd", "rk"):
        tl = t["dnw"][nm]
        for m in range(len(tl)):
            nc.sync.dma_start(
                tl[m][:].rearrange("p (e c) -> p e c", e=NE),
                d[dmap[nm]][:, m * 128:(m + 1) * 128].rearrange(
                    "(e p) c -> p e c", p=128))
    umap = {"ku": "wkuT", "qu": "wquT", "rq": "wrqT", "vu": "wvuT"}
    for nm in ("ku", "qu", "rq", "vu"):
        tl = t["upw"][nm]
        for l in range(NL):
            nc.sync.dma_start(tl[l][:], d[umap[nm]][l * 128:(l + 1) * 128, :])
    for nm, key in (("ones", "onesd"), ("bkvd", "bkvd"), ("bqd", "bqd"),
                    ("bku", "bku"), ("bqu", "bqu"), ("brk", "brk"),
                    ("brq", "brq"), ("cos", "cosT"), ("sin", "sinT"),
                    ("bvu_row", "bvu")):
        nc.sync.dma_start(t[nm][:], d[key][:])
    nc.gpsimd.partition_broadcast(t["bvu_bc"][:], t["bvu_row"][:])


def _load_wo(nc, t, d):
    for hc in range(HPC):
        nc.sync.dma_start(t["wo"][hc][:], d["woT"][hc * 128:(hc + 1) * 128, :])


def _phaseA_pools(tc, pa):
    p = {}
    p["xa"] = pa.enter_context(tc.tile_pool(name="xa", bufs=2))
    p["kvq"] = pa.enter_context(tc.tile_pool(name="kvq", bufs=1))
    p["rp"] = pa.enter_context(tc.tile_pool(name="rp", bufs=2))
    p["psA"] = pa.enter_context(tc.tile_pool(name="psA", bufs=6,
                                             space="PSUM"))
    return p


def _load_x_chunk(nc, d, p, sc):
    import concourse.mybir as mybir
    BF16 = mybir.dt.bfloat16
    ssl = slice(sc * SW, (sc + 1) * SW)
    xt = p["xa"].tile([128, NE * SW], BF16, name="xt")
    nc.sync.dma_start(
        xt[:].rearrange("p (e s) -> p e s", e=NE),
        d["xT"][:, ssl].rearrange("(e p) s -> p e s", p=128))
    return xt


def _emit_A(nc, tc, d, t, p, xts_pre=()):
    import concourse.mybir as mybir
    from concourse.alu_op_type import AluOpType
    F32 = mybir.dt.float32
    BF16 = mybir.dt.bfloat16
    K_t, Q_t, V_t, upw, dnw = t["K"], t["Q"], t["V"], t["upw"], t["dnw"]
    swap_mask = [i ^ 1 for i in range(32)]

    for sc in range(NSC):
        ssl = slice(sc * SW, (sc + 1) * SW)
        xt = (xts_pre[sc] if sc < len(xts_pre)
              else _load_x_chunk(nc, d, p, sc))
        cos_s = t["cos"][:, ssl]
        sin_s = t["sin"][:, ssl]

        def down_mm(wt, m):
            ps = p["psA"].tile([128, SW], F32, name="psA_t")
            for e in range(NE):
                nc.tensor.matmul(ps[:], wt[m][:, e * 128:(e + 1) * 128],
                                 xt[:, e * SW:(e + 1) * SW],
                                 start=(e == 0), stop=(e == NE - 1))
            return ps

        def rope(ps, bias_t, m, dst):
            # ps: [128 rows = 2 heads x 64 rope rows, SW]
            xb = p["rp"].tile([128, SW], BF16, name="xb")
            nc.scalar.add(xb[:], ps[:], bias_t[:, m:m + 1])
            sh = p["rp"].tile([128, SW], BF16, name="sh")
            nc.vector.stream_shuffle(sh[:], xb[:], swap_mask)
            t1 = p["rp"].tile([128, SW], BF16, name="t1")
            nc.vector.tensor_tensor(t1[:], xb[:], cos_s,
                                    op=AluOpType.mult)
            t2 = p["rp"].tile([128, SW], BF16, name="t2")
            nc.vector.tensor_tensor(t2[:], sh[:], sin_s,
                                    op=AluOpType.mult)
            nc.vector.tensor_tensor(dst[2 * m][R:D, ssl], t1[0:R, :],
                                    t2[0:R, :], op=AluOpType.add)
            nc.vector.tensor_tensor(dst[2 * m + 1][R:D, ssl], t1[R:D, :],
                                    t2[R:D, :], op=AluOpType.add)

        def up_mm(src, w, m):
            ps = p["psA"].tile([128, SW], F32, name="psA_t")
            for l in range(NL):
                nc.tensor.matmul(ps[:], w[l][:, m * 128:(m + 1) * 128],
                                 src[l][:], start=(l == 0),
                                 stop=(l == NL - 1))
            return ps

        # latent kv_d down-projection (replicated in batch group)
        kv_s = []
        for m in range(NL):
            ps = down_mm(dnw["kvd"], m)
            tl = p["kvq"].tile([128, SW], BF16, name=f"lat{m}")
            nc.scalar.add(tl[:], ps[:], t["bkvd"][:, m:m + 1])
            kv_s.append(tl)
        for m in range(2):  # k1 -> K rows 0..63
            ps = up_mm(kv_s, upw["ku"], m)
            nc.scalar.add(K_t[2 * m][0:R, ssl], ps[0:R, :],
                          t["bku"][0:R, m:m + 1])
            nc.scalar.add(K_t[2 * m + 1][0:R, ssl], ps[R:D, :],
                          t["bku"][R:D, m:m + 1])
        for j in range(SW // 128):  # V, (s, feat) layout
            ps = p["psA"].tile([128, HPC * D], F32, name="psA_t")
            for l in range(NL):
                nc.tensor.matmul(ps[:], kv_s[l][:, j * 128:(j + 1) * 128],
                                 upw["vu"][l][:], start=(l == 0),
                                 stop=(l == NL - 1))
            nc.vector.tensor_tensor(V_t[sc * (SW // 128) + j][:], ps[:],
                                    t["bvu_bc"][:], op=AluOpType.add)

        # latent q_d down-projection (slots shared with kv_s)
        q_s = []
        for m in range(NL):
            ps = down_mm(dnw["qd"], m)
            tl = p["kvq"].tile([128, SW], BF16, name=f"lat{m}")
            nc.scalar.add(tl[:], ps[:], t["bqd"][:, m:m + 1])
            q_s.append(tl)
        for m in range(2):  # q1 -> Q rows 0..63
            ps = up_mm(q_s, upw["qu"], m)
            nc.scalar.add(Q_t[2 * m][0:R, ssl], ps[0:R, :],
                          t["bqu"][0:R, m:m + 1])
            nc.scalar.add(Q_t[2 * m + 1][0:R, ssl], ps[R:D, :],
                          t["bqu"][R:D, m:m + 1])
        for m in range(2):  # rope-q from q_d
            ps = up_mm(q_s, upw["rq"], m)
            rope(ps, t["brq"], m, Q_t)
        # rope-k from x
        for m in range(2):
            ps = down_mm(dnw["rk"], m)
            rope(ps, t["brk"], m, K_t)


def _phaseB_pools(tc, pb):
    p = {}
    p["pe"] = pb.enter_context(tc.tile_pool(name="pe", bufs=3))
    p["ac"] = pb.enter_context(tc.tile_pool(name="ac", bufs=2))
    p["sm"] = pb.enter_context(tc.tile_pool(name="sm", bufs=2))
    p["cb"] = pb.enter_context(tc.tile_pool(name="cb", bufs=2))
    p["psS"] = pb.enter_context(tc.tile_pool(name="psS", bufs=2,
                                             space="PSUM"))
    p["psO"] = pb.enter_context(tc.tile_pool(name="psO", bufs=2,
                                             space="PSUM"))
    return p


def _emit_B(nc, tc, d, t, p, att_t, mode="full"):
    import concourse.mybir as mybir
    from concourse.alu_op_type import AluOpType
    F32 = mybir.dt.float32
    F32R = mybir.dt.float32r
    BF16 = mybir.dt.bfloat16
    AF = mybir.ActivationFunctionType
    K_t, Q_t, V_t = t["K"], t["Q"], t["V"]

    LAG = 3  # PV trails QK/exp by LAG k-chunks so PE never waits on ACT

    for h in range(HPC):
        for qp in range(2):
            qa = slice((2 * qp) * 512, (2 * qp + 1) * 512)
            qb = slice((2 * qp + 1) * 512, (2 * qp + 2) * 512)
            oA = p["psO"].tile([128, 512], F32, name="oA")
            oB = p["psO"].tile([128, 512], F32, name="oB")
            # two interleaved bf16 row-sum chains on DVE (all-16bit ops
            # run at 2x rate, so they keep up with the exp cadence); the
            # fp32 combine for the ones-matmul happens once at the end
            acc0 = p["ac"].tile([128, 1024], BF16, name="acc0")
            acc1 = p["ac"].tile([128, 1024], BF16, name="acc1")
            accf = p["ac"].tile([128, 1024], F32R, name="accf")
            accs = (acc0, acc1)
            pes = {}

            def pv(kk):
                pe = pes.pop(kk)
                nc.tensor.matmul(oA[:], V_t[kk][:, h * D:(h + 1) * D],
                                 pe[:, 0:512], start=(kk == 0),
                                 stop=(kk == NKC - 1))
                nc.tensor.matmul(oB[:], V_t[kk][:, h * D:(h + 1) * D],
                                 pe[:, 512:1024], start=(kk == 0),
                                 stop=(kk == NKC - 1))

            for kk in range(NKC):
                ksl = slice(kk * 128, (kk + 1) * 128)
                pp = p["psS"].tile([128, 1024], F32, name="pp")
                nc.tensor.matmul(pp[:, 0:512], K_t[h][:, ksl], Q_t[h][:, qa],
                                 start=True, stop=True)
                nc.tensor.matmul(pp[:, 512:1024], K_t[h][:, ksl],
                                 Q_t[h][:, qb], start=True, stop=True)
                if mode == "qk":
                    continue
                pe = p["pe"].tile([128, 1024], BF16, name="pet", bufs=5)
                nc.scalar.activation(pe[:], pp[:], AF.Exp, scale=SCALE)
                if mode == "qke":
                    continue
                # row-sum accumulation (keys land on partitions later)
                acc = accs[kk % 2]
                if kk < 2:
                    nc.vector.tensor_copy(acc[:], pe[:])
                else:
                    nc.vector.tensor_tensor(acc[:], pe[:], acc[:],
                                            op=AluOpType.add)
                pes[kk] = pe
                if kk >= LAG:
                    pv(kk - LAG)
            if mode != "full":
                continue
            for kk in range(NKC - LAG, NKC):
                pv(kk)
            # r[q] = sum_p acc[p, q] via ones-matmul; then 1/r broadcast
            nc.vector.tensor_tensor(accf[:], acc0[:], acc1[:],
                                    op=AluOpType.add)
            sums = p["psS"].tile([1, 1024], F32, name="pp")
            nc.tensor.matmul(sums[:, 0:512], t["ones"][:], accf[:, 0:512],
                             start=True, stop=True)
            nc.tensor.matmul(sums[:, 512:1024], t["ones"][:],
                             accf[:, 512:1024], start=True, stop=True)
            ci = p["sm"].tile([1, 1024], F32, name="ci")
            nc.vector.reciprocal_approx_fast(ci[:], sums[:])
            cb = p["cb"].tile([128, 1024], F32, name="cbt")
            nc.gpsimd.partition_broadcast(cb[:], ci[:])
            nc.vector.tensor_tensor(att_t[h][:, qa], oA[:], cb[:, 0:512],
                                    op=AluOpType.mult)
            nc.vector.tensor_tensor(att_t[h][:, qb], oB[:], cb[:, 512:1024],
                                    op=AluOpType.mult)


def _phaseC_pools(tc, pc):
    p = {}
    p["oc"] = pc.enter_context(tc.tile_pool(name="oc", bufs=3))
    p["psC"] = pc.enter_context(tc.tile_pool(name="psC", bufs=4,
                                             space="PSUM"))
    return p


def _emit_C(nc, tc, d, t, p, att_t):
    import concourse.mybir as mybir
    F32 = mybir.dt.float32
    BF16 = mybir.dt.bfloat16
    wo_t = t["wo"]

    for sj in range(S // 128):
        for ocn in range(E // 512):
            ps = p["psC"].tile([128, 512], F32, name="psC_t")
            for hc in range(HPC):
                nc.tensor.matmul(ps[:], att_t[hc][:, sj * 128:(sj + 1) * 128],
                                 wo_t[hc][:, ocn * 512:(ocn + 1) * 512],
                                 start=(hc == 0), stop=(hc == HPC - 1))
            ob = p["oc"].tile([128, 512], BF16, name="ob")
            if ocn % 2 == 0:
                nc.vector.tensor_copy(ob[:], ps[:])
            else:
                nc.scalar.copy(ob[:], ps[:])
            nc.sync.dma_start(
                d["out"][sj * 128:(sj + 1) * 128,
                         ocn * 512:(ocn + 1) * 512], ob[:])


def _build_program(loop=None):
    """loop=None: normal kernel. loop=(phase, n): benchmark variant with a
    hardware For_i loop repeating one phase n times."""
    import concourse.bacc as bacc
    import concourse.mybir as mybir
    import concourse.tile as tile

    BF16 = mybir.dt.bfloat16

    nc = bacc.Bacc("TRN2", target_bir_lowering=False, debug=False,
                   num_devices=NCORES)
    d = _mk(nc)

    with tile.TileContext(nc) as tc, ExitStack() as top:
        if loop is None:
            # Allocate persistent pools first (pool release is LIFO), but
            # emit the x-chunk prefetch DMAs BEFORE the weight preloads:
            # the very first matmul needs x chunk 0 + the first kv_d
            # weight tile, nothing else.
            t = _consts(nc, tc, top, d, loads=False)
            with ExitStack() as pa:
                pA = _phaseA_pools(tc, pa)
                xts_pre = [_load_x_chunk(nc, d, pA, sc) for sc in range(2)]
                _consts_load(nc, t, d)
                _emit_A(nc, tc, d, t, pA, xts_pre)
            _load_wo(nc, t, d)  # hide the Wo load under phase B
            with ExitStack() as pb:
                att_pool = pb.enter_context(tc.tile_pool(name="att", bufs=1))
                att_t = [att_pool.tile([128, S], BF16, name=f"att{h}")
                         for h in range(HPC)]
                with ExitStack() as pbi:
                    pB = _phaseB_pools(tc, pbi)
                    _emit_B(nc, tc, d, t, pB, att_t)
                with ExitStack() as pc:
                    pC = _phaseC_pools(tc, pc)
                    _emit_C(nc, tc, d, t, pC, att_t)
        else:
            phase, n = loop
            t = _consts(nc, tc, top, d)

            def _fill(tile_, w):
                nc.sync.dma_start(tile_[:], d["xT"][0:128, 0:w])

            with ExitStack() as ps_:
                if phase == "A":
                    pA = _phaseA_pools(tc, ps_)
                    with tc.For_i(0, n, 1):
                        _emit_A(nc, tc, d, t, pA)
                elif phase.startswith("B"):
                    mode = {"B": "full", "B0": "qk", "B1": "qke"}[phase]
                    for h in range(HPC):
                        _fill(t["K"][h], S)
                        _fill(t["Q"][h], S)
                    for i in range(NKC):
                        _fill(t["V"][i], HPC * D)
                    att_pool = ps_.enter_context(
                        tc.tile_pool(name="att", bufs=1))
                    att_t = [att_pool.tile([128, S], BF16, name=f"att{h}")
                             for h in range(HPC)]
                    pB = _phaseB_pools(tc, ps_)
                    with tc.For_i(0, n, 1):
                        _emit_B(nc, tc, d, t, pB, att_t, mode)
                elif phase == "C":
                    att_pool = ps_.enter_context(
                        tc.tile_pool(name="att", bufs=1))
                    att_t = [att_pool.tile([128, S], BF16, name=f"att{h}")
                             for h in range(HPC)]
                    for h in range(HPC):
                        _fill(att_t[h], S)
                    _load_wo(nc, t, d)
                    pC = _phaseC_pools(tc, ps_)
                    with tc.For_i(0, n, 1):
                        _emit_C(nc, tc, d, t, pC, att_t)
                else:
                    raise ValueError(phase)

    nc.compile()
    return nc


def _bf16(a):
    import ml_dtypes
    return np.ascontiguousarray(np.asarray(a, dtype=np.float32)).astype(
        ml_dtypes.bfloat16)


def _rope_tables():
    inv_freq = 1.0 / (10000.0 ** (np.arange(0, R, 2, dtype=np.float64) / R))
    t = np.arange(S, dtype=np.float64)
    freqs = np.outer(t, inv_freq)                       # (S, R/2)
    emb = np.concatenate([freqs, freqs], axis=-1)       # (S, R)
    cos = np.cos(emb).astype(np.float32)                # (S, R)
    sin = np.sin(emb).astype(np.float32)
    perm = np.array([(j // 2) if j % 2 == 0 else (j // 2) + R // 2
                     for j in range(R)])
    sign = np.array([-1.0 if j % 2 == 0 else 1.0
                     for j in range(R)], dtype=np.float32)
    cos_p = cos[:, perm].T.copy()                       # (R, S)
    sin_p = (sin[:, perm] * sign[None, :]).T.copy()     # (R, S)
    cosT = np.concatenate([cos_p, cos_p], axis=0)       # (128, S)
    sinT = np.concatenate([sin_p, sin_p], axis=0)
    return cosT, sinT, perm


def _per_core_inputs(inputs, core):
    b, hg = divmod(core, HPC)
    cosT, sinT, perm = _rope_tables()
    hsl64 = np.concatenate([hg * HPC * R + h * R + perm
                            for h in range(HPC)])       # permuted rope rows
    hs64 = slice(hg * HPC * R, (hg + 1) * HPC * R)      # natural 64-rows
    hs128 = slice(hg * HPC * D, (hg + 1) * HPC * D)     # natural 128-rows

    x = np.asarray(inputs["x"], dtype=np.float32)
    f = np.float32
    im = {
        "xT": _bf16(x[b].T),
        "wkvdT": _bf16(np.asarray(inputs["Wkv_d"], f).T),
        "wqdT": _bf16(np.asarray(inputs["Wq_d"], f).T),
        "wrkT": _bf16(np.asarray(inputs["Wrk"], f)[hsl64].T),
        "wkuT": _bf16(np.asarray(inputs["Wk_u"], f)[hs64].T),
        "wquT": _bf16(np.asarray(inputs["Wq_u"], f)[hs64].T),
        "wrqT": _bf16(np.asarray(inputs["Wrq"], f)[hsl64].T),
        "wvuT": _bf16(np.asarray(inputs["Wv_u"], f)[hs128].T),
        "woT": _bf16(np.asarray(inputs["Wo"], f).T[hs128]),
        "bkvd": np.ascontiguousarray(
            np.asarray(inputs["bkv_d"], f).reshape(NL, 128).T),
        "bqd": np.ascontiguousarray(
            np.asarray(inputs["bq_d"], f).reshape(NL, 128).T),
        "bku": np.ascontiguousarray(
            np.asarray(inputs["bk_u"], f)[hs64].reshape(2, 128).T),
        "bqu": np.ascontiguousarray(
            np.asarray(inputs["bq_u"], f)[hs64].reshape(2, 128).T),
        "brk": np.ascontiguousarray(
            np.asarray(inputs["brk"], f)[hsl64].reshape(2, 128).T),
        "brq": np.ascontiguousarray(
            np.asarray(inputs["brq"], f)[hsl64].reshape(2, 128).T),
        "bvu": np.ascontiguousarray(
            np.asarray(inputs["bv_u"], f)[hs128].reshape(1, HPC * D)),
        "onesd": np.ones((128, 1), dtype=np.float32),
        "cosT": _bf16(cosT),
        "sinT": _bf16(sinT),
    }
    return im


def _get_runtime(loop=None):
    key = loop
    if key in _RT:
        return _RT[key]
    import jax
    import numpy as _np
    from jax.sharding import Mesh, PartitionSpec
    from jax.experimental.shard_map import shard_map

    import concourse.mybir as mybir
    from concourse import bass2jax

    nc = _build_program(loop)
    bass2jax.install_neuronx_cc_hook()

    partition_name = (nc.partition_id_tensor.name
                      if nc.partition_id_tensor else None)
    in_names, out_names, out_avals, zero_shapes = [], [], [], []
    for alloc in nc.m.functions[0].allocations:
        if not isinstance(alloc, mybir.MemoryLocationSet):
            continue
        name = alloc.memorylocations[0].name
        if alloc.kind == "ExternalInput":
            if name != partition_name:
                in_names.append(name)
        elif alloc.kind == "ExternalOutput":
            out_names.append(name)
            np_dt = mybir.dt.np(alloc.dtype)
            out_avals.append(jax.core.ShapedArray(
                tuple(alloc.tensor_shape), np_dt))
            zero_shapes.append((tuple(alloc.tensor_shape), np_dt))

    n_params = len(in_names)
    n_outs = len(out_names)
    all_in_names = list(in_names) + list(out_names)
    if partition_name is not None:
        all_in_names.append(partition_name)

    def _body(*args):
        operands = list(args)
        if partition_name is not None:
            operands.append(bass2jax.partition_id_tensor())
        outs = bass2jax._bass_exec_p.bind(
            *operands,
            out_avals=tuple(out_avals),
            in_names=tuple(all_in_names),
            out_names=tuple(out_names),
            lowering_input_output_aliases=(),
            sim_require_finite=True,
            sim_require_nnan=True,
            nc=nc,
        )
        return tuple(outs)

    devices = jax.devices()[:NCORES]
    mesh = Mesh(_np.asarray(devices), ("core",))
    in_specs = (PartitionSpec("core"),) * (n_params + n_outs)
    out_specs = (PartitionSpec("core"),) * n_outs
    donate = tuple(range(n_params, n_params + n_outs))
    sharded = jax.jit(
        shard_map(_body, mesh=mesh, in_specs=in_specs, out_specs=out_specs,
                  check_rep=False),
        donate_argnums=donate, keep_unused=True)

    _RT[key] = dict(sharded=sharded, in_names=in_names, out_names=out_names,
                    zero_shapes=zero_shapes, n_outs=n_outs, nc=nc)
    return _RT[key]


def _run_cores(in_maps):
    rt = _get_runtime()
    import numpy as _np
    concat_in = [
        _np.concatenate([in_maps[c][name] for c in range(NCORES)], axis=0)
        for name in rt["in_names"]
    ]
    concat_zeros = [
        _np.zeros((NCORES * shp[0],) + shp[1:], dt)
        for (shp, dt) in rt["zero_shapes"]
    ]
    out_arrs = rt["sharded"](*concat_in, *concat_zeros)
    res = []
    for c in range(NCORES):
        m = {}
        for i, name in enumerate(rt["out_names"]):
            shp, dt = rt["zero_shapes"][i]
            m[name] = _np.asarray(out_arrs[i]).reshape((NCORES,) + shp)[c]
        res.append(m)
    return res


def kernel(**inputs):
    in_maps = [_per_core_inputs(inputs, c) for c in range(NCORES)]
    res = _run_cores(in_maps)
    bo = np.asarray(inputs["bo"], dtype=np.float32)
    final = np.empty((B, S, E), dtype=np.float32)
    for b in range(B):
        acc = res[HPC * b]["out"].astype(np.float32).copy()
        for g in range(1, HPC):
            acc += res[HPC * b + g]["out"]
        final[b] = acc + bo[None, :]
    return final


# revision 23
# speedup vs baseline: 1.0219x; 1.0219x over previous
"""MultiHeadLatentAttention TRN2 kernel (v2, bf16).

Sharding: 8 cores = 2 (batch) x 4 (head groups of 4 heads).
Each core computes, for its batch b and heads hg*4..hg*4+3:
  - latent down-projections kv_d, q_d (replicated within the batch group)
  - per-head up-projections K^T, Q^T (with RoPE), V
  - full attention for its 4 heads
  - partial output projection (its 512 columns of Wo's input dim)
Partial outputs are summed on the host (+ bo).

Optimizations vs the first working version (697us/core -> 463us/core on
HW, NTFF-profiled):
  - all matmul operands in bf16 (tolerance test: full-pipeline max-rel
    error ~3e-3 vs the 2e-2 gate); halves LDWEIGHTS + SBUF + DMA.
  - all weights + cos/sin resident in SBUF, loaded once (v1 reloaded
    down-proj weights every s-chunk: ~30MB of redundant DMA); x chunks
    double-buffered and prefetched ahead of the weight preloads so the
    first matmul starts early.
  - softmax row-sum accumulation as two interleaved bf16 chains on DVE
    (all-16-bit DVE ops run at 2x; the Pool engine measured ~2.5ns/elem
    and can't read PSUM, so it only does the 1/sum partition-broadcast);
    1/sum via reciprocal_approx_fast; PSUM double-buffered so the
    softmax epilogue never blocks the next head's QK matmuls.
  - output written in bf16 (halves the HBM write traffic that stalled
    the phase-C PSUM drain).
Phase floors per core: A (projections) ~175us PE-bound, B (attention)
~190us ACT-bound (16.8M exp elements at 0.833ns/elem), C (out-proj)
~65us PE-bound.
"""

import sys

sys.path.insert(0, "/opt/trn_rl_repo")

from contextlib import ExitStack

import numpy as np

H = 16
E = 2048
LAT = E // 4          # 512
D = E // H            # 128
R = D // 2            # 64
B, S = 2, 2048
HPC = H // 4          # 4 heads per core
NCORES = 8
NE = E // 128         # 16 contraction chunks over E
NL = LAT // 128       # 4 contraction chunks over LAT
SW = 512              # s-chunk width for projections
NSC = S // SW         # 4 s-chunks
NKC = S // 128        # 16 key chunks
SCALE = 1.0 / float(np.sqrt(D))

_RT = {}  # cached runtimes


def _mk(nc):
    """Declare DRAM I/O; returns dict of handles."""
    import concourse.mybir as mybir
    F32 = mybir.dt.float32
    F32R = mybir.dt.float32r
    BF16 = mybir.dt.bfloat16
    d = {}
    d["xT"] = nc.dram_tensor("xT", [E, S], BF16, kind="ExternalInput")
    d["wkvdT"] = nc.dram_tensor("wkvdT", [E, LAT], BF16, kind="ExternalInput")
    d["wqdT"] = nc.dram_tensor("wqdT", [E, LAT], BF16, kind="ExternalInput")
    d["wrkT"] = nc.dram_tensor("wrkT", [E, HPC * R], BF16,
                               kind="ExternalInput")
    d["wkuT"] = nc.dram_tensor("wkuT", [LAT, HPC * R], BF16,
                               kind="ExternalInput")
    d["wquT"] = nc.dram_tensor("wquT", [LAT, HPC * R], BF16,
                               kind="ExternalInput")
    d["wrqT"] = nc.dram_tensor("wrqT", [LAT, HPC * R], BF16,
                               kind="ExternalInput")
    d["wvuT"] = nc.dram_tensor("wvuT", [LAT, HPC * D], BF16,
                               kind="ExternalInput")
    d["woT"] = nc.dram_tensor("woT", [HPC * D, E], BF16,
                              kind="ExternalInput")
    d["bkvd"] = nc.dram_tensor("bkvd", [128, NL], F32, kind="ExternalInput")
    d["bqd"] = nc.dram_tensor("bqd", [128, NL], F32, kind="ExternalInput")
    d["bku"] = nc.dram_tensor("bku", [128, 2], F32, kind="ExternalInput")
    d["bqu"] = nc.dram_tensor("bqu", [128, 2], F32, kind="ExternalInput")
    d["brk"] = nc.dram_tensor("brk", [128, 2], F32, kind="ExternalInput")
    d["brq"] = nc.dram_tensor("brq", [128, 2], F32, kind="ExternalInput")
    d["bvu"] = nc.dram_tensor("bvu", [1, HPC * D], F32, kind="ExternalInput")
    d["onesd"] = nc.dram_tensor("onesd", [128, 1], F32R,
                                kind="ExternalInput")
    d["cosT"] = nc.dram_tensor("cosT", [128, S], BF16, kind="ExternalInput")
    d["sinT"] = nc.dram_tensor("sinT", [128, S], BF16, kind="ExternalInput")
    d["out"] = nc.dram_tensor("out", [S, E], BF16, kind="ExternalOutput")
    return d


def _consts(nc, tc, top, d, loads=True):
    """Persistent tiles: K/Q/V storage, biases, ones, all weights.
    With loads=False only allocates; call _consts_load to emit DMAs."""
    import concourse.mybir as mybir
    F32 = mybir.dt.float32
    F32R = mybir.dt.float32r
    BF16 = mybir.dt.bfloat16

    kq_pool = top.enter_context(tc.tile_pool(name="kq", bufs=1))
    v_pool = top.enter_context(tc.tile_pool(name="vp", bufs=1))
    cpool = top.enter_context(tc.tile_pool(name="cp", bufs=1))

    t = {}
    t["K"] = [kq_pool.tile([128, S], BF16, name=f"Kt{h}") for h in range(HPC)]
    t["Q"] = [kq_pool.tile([128, S], BF16, name=f"Qt{h}") for h in range(HPC)]
    t["V"] = [v_pool.tile([128, HPC * D], BF16, name=f"Vt{i}")
              for i in range(NKC)]

    def al(name, shape, dt=F32):
        return cpool.tile(shape, dt, name=name)

    # Down-projection weights: per output m-chunk, a [128, NE*128] tile
    # whose e-th column block is the lhsT for contraction chunk e.
    t["dnw"] = {
        nm: [al(f"wd_{nm}{m}", [128, NE * 128], BF16)
             for m in range(nm_chunks)]
        for nm, nm_chunks in (("kvd", NL), ("qd", NL), ("rk", 2))
    }
    t["upw"] = {
        nm: [al(f"w{nm}{l}", [128, w], BF16) for l in range(NL)]
        for nm, w in (("ku", HPC * R), ("qu", HPC * R),
                      ("rq", HPC * R), ("vu", HPC * D))
    }
    t["ones"] = al("ones_t", [128, 1], F32R)
    t["bkvd"] = al("bkvd_t", [128, NL])
    t["bqd"] = al("bqd_t", [128, NL])
    t["bku"] = al("bku_t", [128, 2])
    t["bqu"] = al("bqu_t", [128, 2])
    t["brk"] = al("brk_t", [128, 2])
    t["brq"] = al("brq_t", [128, 2])
    t["cos"] = al("cos_t", [128, S], BF16)
    t["sin"] = al("sin_t", [128, S], BF16)
    t["bvu_row"] = al("bvu_row", [1, HPC * D])
    t["bvu_bc"] = al("bvu_bc", [128, HPC * D])
    # wo loads are issued later (before phase C) via _load_wo
    t["wo"] = [al(f"wo{hc}", [128, E], BF16) for hc in range(HPC)]
    if loads:
        _consts_load(nc, t, d)
    return t


def _consts_load_down(nc, t, d):
    """Down-projection weight DMAs (phase A's matmuls need these first)."""
    dmap = {"kvd": "wkvdT", "qd": "wqdT", "rk": "wrkT"}
    for nm in ("kvd", "qd", "rk"):
        tl = t["dnw"][nm]
        for m in range(len(tl)):
            nc.sync.dma_start(
                tl[m][:].rearrange("p (e c) -> p e c", e=NE),
                d[dmap[nm]][:, m * 128:(m + 1) * 128].rearrange(
                    "(e p) c -> p e c", p=128))


def _consts_load_rest(nc, t, d):
    umap = {"ku": "wkuT", "qu": "wquT", "rq": "wrqT", "vu": "wvuT"}
    for nm in ("ku", "qu", "rq", "vu"):
        tl = t["upw"][nm]
        for l in range(NL):
            nc.sync.dma_start(tl[l][:], d[umap[nm]][l * 128:(l + 1) * 128, :])
    for nm, key in (("ones", "onesd"), ("bkvd", "bkvd"), ("bqd", "bqd"),
                    ("bku", "bku"), ("bqu", "bqu"), ("brk", "brk"),
                    ("brq", "brq"), ("cos", "cosT"), ("sin", "sinT"),
                    ("bvu_row", "bvu")):
        nc.sync.dma_start(t[nm][:], d[key][:])
    nc.gpsimd.partition_broadcast(t["bvu_bc"][:], t["bvu_row"][:])


def _consts_load(nc, t, d):
    _consts_load_down(nc, t, d)
    _consts_load_rest(nc, t, d)


def _load_wo(nc, t, d):
    for hc in range(HPC):
        nc.sync.dma_start(t["wo"][hc][:], d["woT"][hc * 128:(hc + 1) * 128, :])


def _phaseA_pools(tc, pa):
    p = {}
    p["xa"] = pa.enter_context(tc.tile_pool(name="xa", bufs=2))
    p["kvq"] = pa.enter_context(tc.tile_pool(name="kvq", bufs=1))
    p["rp"] = pa.enter_context(tc.tile_pool(name="rp", bufs=2))
    p["psA"] = pa.enter_context(tc.tile_pool(name="psA", bufs=6,
                                             space="PSUM"))
    return p


def _load_x_chunk(nc, d, p, sc, eng=None):
    import concourse.mybir as mybir
    BF16 = mybir.dt.bfloat16
    ssl = slice(sc * SW, (sc + 1) * SW)
    xt = p["xa"].tile([128, NE * SW], BF16, name="xt")
    (eng or nc.sync).dma_start(
        xt[:].rearrange("p (e s) -> p e s", e=NE),
        d["xT"][:, ssl].rearrange("(e p) s -> p e s", p=128))
    return xt


def _emit_A(nc, tc, d, t, p, xts_pre=()):
    import concourse.mybir as mybir
    from concourse.alu_op_type import AluOpType
    F32 = mybir.dt.float32
    BF16 = mybir.dt.bfloat16
    K_t, Q_t, V_t, upw, dnw = t["K"], t["Q"], t["V"], t["upw"], t["dnw"]
    swap_mask = [i ^ 1 for i in range(32)]

    for sc in range(NSC):
        ssl = slice(sc * SW, (sc + 1) * SW)
        xt = (xts_pre[sc] if sc < len(xts_pre)
              else _load_x_chunk(nc, d, p, sc))
        cos_s = t["cos"][:, ssl]
        sin_s = t["sin"][:, ssl]

        def down_mm(wt, m):
            ps = p["psA"].tile([128, SW], F32, name="psA_t")
            for e in range(NE):
                nc.tensor.matmul(ps[:], wt[m][:, e * 128:(e + 1) * 128],
                                 xt[:, e * SW:(e + 1) * SW],
                                 start=(e == 0), stop=(e == NE - 1))
            return ps

        def rope(ps, bias_t, m, dst):
            # ps: [128 rows = 2 heads x 64 rope rows, SW]
            xb = p["rp"].tile([128, SW], BF16, name="xb")
            nc.scalar.add(xb[:], ps[:], bias_t[:, m:m + 1])
            sh = p["rp"].tile([128, SW], BF16, name="sh")
            nc.vector.stream_shuffle(sh[:], xb[:], swap_mask)
            t1 = p["rp"].tile([128, SW], BF16, name="t1")
            nc.vector.tensor_tensor(t1[:], xb[:], cos_s,
                                    op=AluOpType.mult)
            t2 = p["rp"].tile([128, SW], BF16, name="t2")
            nc.vector.tensor_tensor(t2[:], sh[:], sin_s,
                                    op=AluOpType.mult)
            nc.vector.tensor_tensor(dst[2 * m][R:D, ssl], t1[0:R, :],
                                    t2[0:R, :], op=AluOpType.add)
            nc.vector.tensor_tensor(dst[2 * m + 1][R:D, ssl], t1[R:D, :],
                                    t2[R:D, :], op=AluOpType.add)

        def up_mm(src, w, m):
            ps = p["psA"].tile([128, SW], F32, name="psA_t")
            for l in range(NL):
                nc.tensor.matmul(ps[:], w[l][:, m * 128:(m + 1) * 128],
                                 src[l][:], start=(l == 0),
                                 stop=(l == NL - 1))
            return ps

        # latent kv_d down-projection (replicated in batch group)
        kv_s = []
        for m in range(NL):
            ps = down_mm(dnw["kvd"], m)
            tl = p["kvq"].tile([128, SW], BF16, name=f"lat{m}")
            nc.scalar.add(tl[:], ps[:], t["bkvd"][:, m:m + 1])
            kv_s.append(tl)
        for m in range(2):  # k1 -> K rows 0..63
            ps = up_mm(kv_s, upw["ku"], m)
            nc.scalar.add(K_t[2 * m][0:R, ssl], ps[0:R, :],
                          t["bku"][0:R, m:m + 1])
            nc.scalar.add(K_t[2 * m + 1][0:R, ssl], ps[R:D, :],
                          t["bku"][R:D, m:m + 1])
        for j in range(SW // 128):  # V, (s, feat) layout
            ps = p["psA"].tile([128, HPC * D], F32, name="psA_t")
            for l in range(NL):
                nc.tensor.matmul(ps[:], kv_s[l][:, j * 128:(j + 1) * 128],
                                 upw["vu"][l][:], start=(l == 0),
                                 stop=(l == NL - 1))
            nc.vector.tensor_tensor(V_t[sc * (SW // 128) + j][:], ps[:],
                                    t["bvu_bc"][:], op=AluOpType.add)

        # latent q_d down-projection (slots shared with kv_s)
        q_s = []
        for m in range(NL):
            ps = down_mm(dnw["qd"], m)
            tl = p["kvq"].tile([128, SW], BF16, name=f"lat{m}")
            nc.scalar.add(tl[:], ps[:], t["bqd"][:, m:m + 1])
            q_s.append(tl)
        for m in range(2):  # q1 -> Q rows 0..63
            ps = up_mm(q_s, upw["qu"], m)
            nc.scalar.add(Q_t[2 * m][0:R, ssl], ps[0:R, :],
                          t["bqu"][0:R, m:m + 1])
            nc.scalar.add(Q_t[2 * m + 1][0:R, ssl], ps[R:D, :],
                          t["bqu"][R:D, m:m + 1])
        for m in range(2):  # rope-q from q_d
            ps = up_mm(q_s, upw["rq"], m)
            rope(ps, t["brq"], m, Q_t)
        # rope-k from x
        for m in range(2):
            ps = down_mm(dnw["rk"], m)
            rope(ps, t["brk"], m, K_t)


def _phaseB_pools(tc, pb):
    p = {}
    p["pe"] = pb.enter_context(tc.tile_pool(name="pe", bufs=3))
    p["ac"] = pb.enter_context(tc.tile_pool(name="ac", bufs=2))
    p["sm"] = pb.enter_context(tc.tile_pool(name="sm", bufs=2))
    p["cb"] = pb.enter_context(tc.tile_pool(name="cb", bufs=2))
    p["psS"] = pb.enter_context(tc.tile_pool(name="psS", bufs=2,
                                             space="PSUM"))
    p["psO"] = pb.enter_context(tc.tile_pool(name="psO", bufs=1,
                                             space="PSUM"))
    p["psR"] = pb.enter_context(tc.tile_pool(name="psR", bufs=2,
                                             space="PSUM"))
    return p


def _emit_B(nc, tc, d, t, p, att_t, mode="full"):
    import concourse.mybir as mybir
    from concourse.alu_op_type import AluOpType
    F32 = mybir.dt.float32
    F32R = mybir.dt.float32r
    BF16 = mybir.dt.bfloat16
    AF = mybir.ActivationFunctionType
    K_t, Q_t, V_t = t["K"], t["Q"], t["V"]

    LAG = 3  # PV trails QK/exp by LAG k-chunks so PE never waits on ACT

    for h in range(HPC):
        for qp in range(2):
            qa = slice((2 * qp) * 512, (2 * qp + 1) * 512)
            qb = slice((2 * qp + 1) * 512, (2 * qp + 2) * 512)
            oA = p["psO"].tile([128, 512], F32, name="oA")
            oB = p["psO"].tile([128, 512], F32, name="oB")
            # two interleaved bf16 row-sum chains on DVE (all-16bit ops
            # run at 2x rate, so they keep up with the exp cadence); the
            # fp32 combine for the ones-matmul happens once at the end
            acc0 = p["ac"].tile([128, 1024], BF16, name="acc0")
            acc1 = p["ac"].tile([128, 1024], BF16, name="acc1")
            accf = p["ac"].tile([128, 1024], F32R, name="accf")
            accs = (acc0, acc1)
            pes = {}

            def pv(kk):
                pe = pes.pop(kk)
                nc.tensor.matmul(oA[:], V_t[kk][:, h * D:(h + 1) * D],
                                 pe[:, 0:512], start=(kk == 0),
                                 stop=(kk == NKC - 1))
                nc.tensor.matmul(oB[:], V_t[kk][:, h * D:(h + 1) * D],
                                 pe[:, 512:1024], start=(kk == 0),
                                 stop=(kk == NKC - 1))

            for kk in range(NKC):
                ksl = slice(kk * 128, (kk + 1) * 128)
                pp = p["psS"].tile([128, 1024], F32, name="pp")
                nc.tensor.matmul(pp[:, 0:512], K_t[h][:, ksl], Q_t[h][:, qa],
                                 start=True, stop=True)
                nc.tensor.matmul(pp[:, 512:1024], K_t[h][:, ksl],
                                 Q_t[h][:, qb], start=True, stop=True)
                if mode == "qk":
                    continue
                pe = p["pe"].tile([128, 1024], BF16, name="pet", bufs=5)
                nc.scalar.activation(pe[:], pp[:], AF.Exp, scale=SCALE)
                if mode == "qke":
                    continue
                # row-sum accumulation (keys land on partitions later)
                acc = accs[kk % 2]
                if kk < 2:
                    nc.vector.tensor_copy(acc[:], pe[:])
                else:
                    nc.vector.tensor_tensor(acc[:], pe[:], acc[:],
                                            op=AluOpType.add)
                pes[kk] = pe
                if kk >= LAG:
                    pv(kk - LAG)
            if mode != "full":
                continue
            for kk in range(NKC - LAG, NKC):
                pv(kk)
            # r[q] = sum_p acc[p, q] via ones-matmul; then 1/r broadcast
            nc.vector.tensor_tensor(accf[:], acc0[:], acc1[:],
                                    op=AluOpType.add)
            # sums live in their own PSUM pool (NOT the pp pool): when
            # they shared pp's slots, the next iteration's QK matmuls
            # waited on the reciprocal to drain the slot, starving the
            # Activation engine ~3.5us per head-block.
            ci = p["sm"].tile([1, 1024], F32, name="ci")
            for half in range(2):
                hs = slice(half * 512, (half + 1) * 512)
                sums = p["psR"].tile([1, 512], F32, name="sums")
                nc.tensor.matmul(sums[:], t["ones"][:], accf[:, hs],
                                 start=True, stop=True)
                nc.vector.reciprocal_approx_fast(ci[:, hs], sums[:])
            cb = p["cb"].tile([128, 1024], F32, name="cbt")
            nc.gpsimd.partition_broadcast(cb[:], ci[:])
            nc.vector.tensor_tensor(att_t[h][:, qa], oA[:], cb[:, 0:512],
                                    op=AluOpType.mult)
            nc.vector.tensor_tensor(att_t[h][:, qb], oB[:], cb[:, 512:1024],
                                    op=AluOpType.mult)


def _phaseC_pools(tc, pc):
    p = {}
    p["oc"] = pc.enter_context(tc.tile_pool(name="oc", bufs=3))
    p["psC"] = pc.enter_context(tc.tile_pool(name="psC", bufs=4,
                                             space="PSUM"))
    return p


def _emit_C(nc, tc, d, t, p, att_t):
    import concourse.mybir as mybir
    F32 = mybir.dt.float32
    BF16 = mybir.dt.bfloat16
    wo_t = t["wo"]

    for sj in range(S // 128):
        for ocn in range(E // 512):
            ps = p["psC"].tile([128, 512], F32, name="psC_t")
            for hc in range(HPC):
                nc.tensor.matmul(ps[:], att_t[hc][:, sj * 128:(sj + 1) * 128],
                                 wo_t[hc][:, ocn * 512:(ocn + 1) * 512],
                                 start=(hc == 0), stop=(hc == HPC - 1))
            ob = p["oc"].tile([128, 512], BF16, name="ob")
            nc.vector.tensor_copy(ob[:], ps[:])
            # alternate DMA issue queues: a single queue's ~850ns issue
            # cost per tile would gate the drain of 64 output tiles
            eng = nc.sync if ocn % 2 == 0 else nc.scalar
            eng.dma_start(
                d["out"][sj * 128:(sj + 1) * 128,
                         ocn * 512:(ocn + 1) * 512], ob[:])


def _build_program(loop=None):
    """loop=None: normal kernel. loop=(phase, n): benchmark variant with a
    hardware For_i loop repeating one phase n times."""
    import concourse.bacc as bacc
    import concourse.mybir as mybir
    import concourse.tile as tile

    BF16 = mybir.dt.bfloat16

    nc = bacc.Bacc("TRN2", target_bir_lowering=False, debug=False,
                   num_devices=NCORES)
    d = _mk(nc)

    with tile.TileContext(nc) as tc, ExitStack() as top:
        if loop is None:
            # Allocate persistent pools first (pool release is LIFO), and
            # stagger the preload DMA queue by first-use so nothing the
            # first ~50us of compute needs sits behind 10MB of weights:
            # x0, down-weights, x1, everything else.  x2/x3 prefetch on
            # the (otherwise idle in phase A) gpsimd queue, where their
            # wait for an xa slot blocks nothing.
            t = _consts(nc, tc, top, d, loads=False)
            with ExitStack() as pa:
                pA = _phaseA_pools(tc, pa)
                xts_pre = [_load_x_chunk(nc, d, pA, 0)]
                _consts_load_down(nc, t, d)
                xts_pre.append(_load_x_chunk(nc, d, pA, 1))
                _consts_load_rest(nc, t, d)
                xts_pre.append(_load_x_chunk(nc, d, pA, 2, eng=nc.gpsimd))
                xts_pre.append(_load_x_chunk(nc, d, pA, 3, eng=nc.gpsimd))
                _emit_A(nc, tc, d, t, pA, xts_pre)
            _load_wo(nc, t, d)  # hide the Wo load under phase B
            with ExitStack() as pb:
                att_pool = pb.enter_context(tc.tile_pool(name="att", bufs=1))
                att_t = [att_pool.tile([128, S], BF16, name=f"att{h}")
                         for h in range(HPC)]
                with ExitStack() as pbi:
                    pB = _phaseB_pools(tc, pbi)
                    _emit_B(nc, tc, d, t, pB, att_t)
                with ExitStack() as pc:
                    pC = _phaseC_pools(tc, pc)
                    _emit_C(nc, tc, d, t, pC, att_t)
        else:
            phase, n = loop
            t = _consts(nc, tc, top, d)

            def _fill(tile_, w):
                nc.sync.dma_start(tile_[:], d["xT"][0:128, 0:w])

            with ExitStack() as ps_:
                if phase == "A":
                    pA = _phaseA_pools(tc, ps_)
                    with tc.For_i(0, n, 1):
                        _emit_A(nc, tc, d, t, pA)
                elif phase.startswith("B"):
                    mode = {"B": "full", "B0": "qk", "B1": "qke"}[phase]
                    for h in range(HPC):
                        _fill(t["K"][h], S)
                        _fill(t["Q"][h], S)
                    for i in range(NKC):
                        _fill(t["V"][i], HPC * D)
                    att_pool = ps_.enter_context(
                        tc.tile_pool(name="att", bufs=1))
                    att_t = [att_pool.tile([128, S], BF16, name=f"att{h}")
                             for h in range(HPC)]
                    pB = _phaseB_pools(tc, ps_)
                    with tc.For_i(0, n, 1):
                        _emit_B(nc, tc, d, t, pB, att_t, mode)
                elif phase == "C":
                    att_pool = ps_.enter_context(
                        tc.tile_pool(name="att", bufs=1))
                    att_t = [att_pool.tile([128, S], BF16, name=f"att{h}")
                             for h in range(HPC)]
                    for h in range(HPC):
                        _fill(att_t[h], S)
                    _load_wo(nc, t, d)
                    pC = _phaseC_pools(tc, ps_)
                    with tc.For_i(0, n, 1):
                        _emit_C(nc, tc, d, t, pC, att_t)
                else:
                    raise ValueError(phase)

    nc.compile()
    return nc


def _bf16(a):
    import ml_dtypes
    return np.ascontiguousarray(np.asarray(a, dtype=np.float32)).astype(
        ml_dtypes.bfloat16)


def _rope_tables():
    inv_freq = 1.0 / (10000.0 ** (np.arange(0, R, 2, dtype=np.float64) / R))
    t = np.arange(S, dtype=np.float64)
    freqs = np.outer(t, inv_freq)                       # (S, R/2)
    emb = np.concatenate([freqs, freqs], axis=-1)       # (S, R)
    cos = np.cos(emb).astype(np.float32)                # (S, R)
    sin = np.sin(emb).astype(np.float32)
    perm = np.array([(j // 2) if j % 2 == 0 else (j // 2) + R // 2
                     for j in range(R)])
    sign = np.array([-1.0 if j % 2 == 0 else 1.0
                     for j in range(R)], dtype=np.float32)
    cos_p = cos[:, perm].T.copy()                       # (R, S)
    sin_p = (sin[:, perm] * sign[None, :]).T.copy()     # (R, S)
    cosT = np.concatenate([cos_p, cos_p], axis=0)       # (128, S)
    sinT = np.concatenate([sin_p, sin_p], axis=0)
    return cosT, sinT, perm


def _per_core_inputs(inputs, core):
    b, hg = divmod(core, HPC)
    cosT, sinT, perm = _rope_tables()
    hsl64 = np.concatenate([hg * HPC * R + h * R + perm
                            for h in range(HPC)])       # permuted rope rows
    hs64 = slice(hg * HPC * R, (hg + 1) * HPC * R)      # natural 64-rows
    hs128 = slice(hg * HPC * D, (hg + 1) * HPC * D)     # natural 128-rows

    x = np.asarray(inputs["x"], dtype=np.float32)
    f = np.float32
    im = {
        "xT": _bf16(x[b].T),
        "wkvdT": _bf16(np.asarray(inputs["Wkv_d"], f).T),
        "wqdT": _bf16(np.asarray(inputs["Wq_d"], f).T),
        "wrkT": _bf16(np.asarray(inputs["Wrk"], f)[hsl64].T),
        "wkuT": _bf16(np.asarray(inputs["Wk_u"], f)[hs64].T),
        "wquT": _bf16(np.asarray(inputs["Wq_u"], f)[hs64].T),
        "wrqT": _bf16(np.asarray(inputs["Wrq"], f)[hsl64].T),
        "wvuT": _bf16(np.asarray(inputs["Wv_u"], f)[hs128].T),
        "woT": _bf16(np.asarray(inputs["Wo"], f).T[hs128]),
        "bkvd": np.ascontiguousarray(
            np.asarray(inputs["bkv_d"], f).reshape(NL, 128).T),
        "bqd": np.ascontiguousarray(
            np.asarray(inputs["bq_d"], f).reshape(NL, 128).T),
        "bku": np.ascontiguousarray(
            np.asarray(inputs["bk_u"], f)[hs64].reshape(2, 128).T),
        "bqu": np.ascontiguousarray(
            np.asarray(inputs["bq_u"], f)[hs64].reshape(2, 128).T),
        "brk": np.ascontiguousarray(
            np.asarray(inputs["brk"], f)[hsl64].reshape(2, 128).T),
        "brq": np.ascontiguousarray(
            np.asarray(inputs["brq"], f)[hsl64].reshape(2, 128).T),
        "bvu": np.ascontiguousarray(
            np.asarray(inputs["bv_u"], f)[hs128].reshape(1, HPC * D)),
        "onesd": np.ones((128, 1), dtype=np.float32),
        "cosT": _bf16(cosT),
        "sinT": _bf16(sinT),
    }
    return im


def _get_runtime(loop=None):
    key = loop
    if key in _RT:
        return _RT[key]
    import jax
    import numpy as _np
    from jax.sharding import Mesh, PartitionSpec
    from jax.experimental.shard_map import shard_map

    import concourse.mybir as mybir
    from concourse import bass2jax

    nc = _build_program(loop)
    bass2jax.install_neuronx_cc_hook()

    partition_name = (nc.partition_id_tensor.name
                      if nc.partition_id_tensor else None)
    in_names, out_names, out_avals, zero_shapes = [], [], [], []
    for alloc in nc.m.functions[0].allocations:
        if not isinstance(alloc, mybir.MemoryLocationSet):
            continue
        name = alloc.memorylocations[0].name
        if alloc.kind == "ExternalInput":
            if name != partition_name:
                in_names.append(name)
        elif alloc.kind == "ExternalOutput":
            out_names.append(name)
            np_dt = mybir.dt.np(alloc.dtype)
            out_avals.append(jax.core.ShapedArray(
                tuple(alloc.tensor_shape), np_dt))
            zero_shapes.append((tuple(alloc.tensor_shape), np_dt))

    n_params = len(in_names)
    n_outs = len(out_names)
    all_in_names = list(in_names) + list(out_names)
    if partition_name is not None:
        all_in_names.append(partition_name)

    def _body(*args):
        operands = list(args)
        if partition_name is not None:
            operands.append(bass2jax.partition_id_tensor())
        outs = bass2jax._bass_exec_p.bind(
            *operands,
            out_avals=tuple(out_avals),
            in_names=tuple(all_in_names),
            out_names=tuple(out_names),
            lowering_input_output_aliases=(),
            sim_require_finite=True,
            sim_require_nnan=True,
            nc=nc,
        )
        return tuple(outs)

    devices = jax.devices()[:NCORES]
    mesh = Mesh(_np.asarray(devices), ("core",))
    in_specs = (PartitionSpec("core"),) * (n_params + n_outs)
    out_specs = (PartitionSpec("core"),) * n_outs
    donate = tuple(range(n_params, n_params + n_outs))
    sharded = jax.jit(
        shard_map(_body, mesh=mesh, in_specs=in_specs, out_specs=out_specs,
                  check_rep=False),
        donate_argnums=donate, keep_unused=True)

    _RT[key] = dict(sharded=sharded, in_names=in_names, out_names=out_names,
                    zero_shapes=zero_shapes, n_outs=n_outs, nc=nc)
    return _RT[key]


def _run_cores(in_maps):
    rt = _get_runtime()
    import numpy as _np
    concat_in = [
        _np.concatenate([in_maps[c][name] for c in range(NCORES)], axis=0)
        for name in rt["in_names"]
    ]
    concat_zeros = [
        _np.zeros((NCORES * shp[0],) + shp[1:], dt)
        for (shp, dt) in rt["zero_shapes"]
    ]
    out_arrs = rt["sharded"](*concat_in, *concat_zeros)
    res = []
    for c in range(NCORES):
        m = {}
        for i, name in enumerate(rt["out_names"]):
            shp, dt = rt["zero_shapes"][i]
            m[name] = _np.asarray(out_arrs[i]).reshape((NCORES,) + shp)[c]
        res.append(m)
    return res


def kernel(**inputs):
    in_maps = [_per_core_inputs(inputs, c) for c in range(NCORES)]
    res = _run_cores(in_maps)
    bo = np.asarray(inputs["bo"], dtype=np.float32)
    final = np.empty((B, S, E), dtype=np.float32)
    for b in range(B):
        acc = res[HPC * b]["out"].astype(np.float32).copy()
        for g in range(1, HPC):
            acc += res[HPC * b + g]["out"]
        final[b] = acc + bo[None, :]
    return final


# revision 24
# speedup vs baseline: 1.0299x; 1.0078x over previous
"""MultiHeadLatentAttention TRN2 kernel (v2, bf16).

Sharding: 8 cores = 2 (batch) x 4 (head groups of 4 heads).
Each core computes, for its batch b and heads hg*4..hg*4+3:
  - latent down-projections kv_d, q_d (replicated within the batch group)
  - per-head up-projections K^T, Q^T (with RoPE), V
  - full attention for its 4 heads
  - partial output projection (its 512 columns of Wo's input dim)
Partial outputs are summed on the host (+ bo).

Optimizations vs the first working version (697us/core -> 463us/core on
HW, NTFF-profiled):
  - all matmul operands in bf16 (tolerance test: full-pipeline max-rel
    error ~3e-3 vs the 2e-2 gate); halves LDWEIGHTS + SBUF + DMA.
  - all weights + cos/sin resident in SBUF, loaded once (v1 reloaded
    down-proj weights every s-chunk: ~30MB of redundant DMA); x chunks
    double-buffered and prefetched ahead of the weight preloads so the
    first matmul starts early.
  - softmax row-sum accumulation as two interleaved bf16 chains on DVE
    (all-16-bit DVE ops run at 2x; the Pool engine measured ~2.5ns/elem
    and can't read PSUM, so it only does the 1/sum partition-broadcast);
    1/sum via reciprocal_approx_fast; PSUM double-buffered so the
    softmax epilogue never blocks the next head's QK matmuls.
  - output written in bf16 (halves the HBM write traffic that stalled
    the phase-C PSUM drain).
Phase floors per core: A (projections) ~175us PE-bound, B (attention)
~190us ACT-bound (16.8M exp elements at 0.833ns/elem), C (out-proj)
~65us PE-bound.
"""

import sys

sys.path.insert(0, "/opt/trn_rl_repo")

from contextlib import ExitStack

import numpy as np

H = 16
E = 2048
LAT = E // 4          # 512
D = E // H            # 128
R = D // 2            # 64
B, S = 2, 2048
HPC = H // 4          # 4 heads per core
NCORES = 8
NE = E // 128         # 16 contraction chunks over E
NL = LAT // 128       # 4 contraction chunks over LAT
SW = 512              # s-chunk width for projections
NSC = S // SW         # 4 s-chunks
NKC = S // 128        # 16 key chunks
SCALE = 1.0 / float(np.sqrt(D))

_RT = {}  # cached runtimes


def _mk(nc):
    """Declare DRAM I/O; returns dict of handles."""
    import concourse.mybir as mybir
    F32 = mybir.dt.float32
    F32R = mybir.dt.float32r
    BF16 = mybir.dt.bfloat16
    d = {}
    d["xT"] = nc.dram_tensor("xT", [E, S], BF16, kind="ExternalInput")
    d["wkvdT"] = nc.dram_tensor("wkvdT", [E, LAT], BF16, kind="ExternalInput")
    d["wqdT"] = nc.dram_tensor("wqdT", [E, LAT], BF16, kind="ExternalInput")
    d["wrkT"] = nc.dram_tensor("wrkT", [E, HPC * R], BF16,
                               kind="ExternalInput")
    d["wkuT"] = nc.dram_tensor("wkuT", [LAT, HPC * R], BF16,
                               kind="ExternalInput")
    d["wquT"] = nc.dram_tensor("wquT", [LAT, HPC * R], BF16,
                               kind="ExternalInput")
    d["wrqT"] = nc.dram_tensor("wrqT", [LAT, HPC * R], BF16,
                               kind="ExternalInput")
    d["wvuT"] = nc.dram_tensor("wvuT", [LAT, HPC * D], BF16,
                               kind="ExternalInput")
    d["woT"] = nc.dram_tensor("woT", [HPC * D, E], BF16,
                              kind="ExternalInput")
    d["bkvd"] = nc.dram_tensor("bkvd", [128, NL], F32, kind="ExternalInput")
    d["bqd"] = nc.dram_tensor("bqd", [128, NL], F32, kind="ExternalInput")
    d["bku"] = nc.dram_tensor("bku", [128, 2], F32, kind="ExternalInput")
    d["bqu"] = nc.dram_tensor("bqu", [128, 2], F32, kind="ExternalInput")
    d["brk"] = nc.dram_tensor("brk", [128, 2], F32, kind="ExternalInput")
    d["brq"] = nc.dram_tensor("brq", [128, 2], F32, kind="ExternalInput")
    d["bvu"] = nc.dram_tensor("bvu", [1, HPC * D], F32, kind="ExternalInput")
    d["onesd"] = nc.dram_tensor("onesd", [128, 1], F32R,
                                kind="ExternalInput")
    d["cosT"] = nc.dram_tensor("cosT", [128, S], BF16, kind="ExternalInput")
    d["sinT"] = nc.dram_tensor("sinT", [128, S], BF16, kind="ExternalInput")
    d["out"] = nc.dram_tensor("out", [S, E], BF16, kind="ExternalOutput")
    return d


def _consts(nc, tc, top, d, loads=True):
    """Persistent tiles: K/Q/V storage, biases, ones, all weights.
    With loads=False only allocates; call _consts_load to emit DMAs."""
    import concourse.mybir as mybir
    F32 = mybir.dt.float32
    F32R = mybir.dt.float32r
    BF16 = mybir.dt.bfloat16

    kq_pool = top.enter_context(tc.tile_pool(name="kq", bufs=1))
    v_pool = top.enter_context(tc.tile_pool(name="vp", bufs=1))
    cpool = top.enter_context(tc.tile_pool(name="cp", bufs=1))

    t = {}
    t["K"] = [kq_pool.tile([128, S], BF16, name=f"Kt{h}") for h in range(HPC)]
    t["Q"] = [kq_pool.tile([128, S], BF16, name=f"Qt{h}") for h in range(HPC)]
    t["V"] = [v_pool.tile([128, HPC * D], BF16, name=f"Vt{i}")
              for i in range(NKC)]

    def al(name, shape, dt=F32):
        return cpool.tile(shape, dt, name=name)

    # Down-projection weights: per output m-chunk, a [128, NE*128] tile
    # whose e-th column block is the lhsT for contraction chunk e.
    t["dnw"] = {
        nm: [al(f"wd_{nm}{m}", [128, NE * 128], BF16)
             for m in range(nm_chunks)]
        for nm, nm_chunks in (("kvd", NL), ("qd", NL), ("rk", 2))
    }
    t["upw"] = {
        nm: [al(f"w{nm}{l}", [128, w], BF16) for l in range(NL)]
        for nm, w in (("ku", HPC * R), ("qu", HPC * R),
                      ("rq", HPC * R), ("vu", HPC * D))
    }
    t["ones"] = al("ones_t", [128, 1], F32R)
    t["bkvd"] = al("bkvd_t", [128, NL])
    t["bqd"] = al("bqd_t", [128, NL])
    t["bku"] = al("bku_t", [128, 2])
    t["bqu"] = al("bqu_t", [128, 2])
    t["brk"] = al("brk_t", [128, 2])
    t["brq"] = al("brq_t", [128, 2])
    t["cos"] = al("cos_t", [128, S], BF16)
    t["sin"] = al("sin_t", [128, S], BF16)
    t["bvu_row"] = al("bvu_row", [1, HPC * D])
    t["bvu_bc"] = al("bvu_bc", [128, HPC * D])
    # wo loads are issued later (before phase C) via _load_wo
    t["wo"] = [al(f"wo{hc}", [128, E], BF16) for hc in range(HPC)]
    if loads:
        _consts_load(nc, t, d)
    return t


def _consts_load_down(nc, t, d):
    """Down-projection weight DMAs (phase A's matmuls need these first)."""
    dmap = {"kvd": "wkvdT", "qd": "wqdT", "rk": "wrkT"}
    for nm in ("kvd", "qd", "rk"):
        tl = t["dnw"][nm]
        for m in range(len(tl)):
            nc.sync.dma_start(
                tl[m][:].rearrange("p (e c) -> p e c", e=NE),
                d[dmap[nm]][:, m * 128:(m + 1) * 128].rearrange(
                    "(e p) c -> p e c", p=128))


def _consts_load_rest(nc, t, d):
    umap = {"ku": "wkuT", "qu": "wquT", "rq": "wrqT", "vu": "wvuT"}
    for nm in ("ku", "qu", "rq", "vu"):
        tl = t["upw"][nm]
        for l in range(NL):
            nc.sync.dma_start(tl[l][:], d[umap[nm]][l * 128:(l + 1) * 128, :])
    for nm, key in (("ones", "onesd"), ("bkvd", "bkvd"), ("bqd", "bqd"),
                    ("bku", "bku"), ("bqu", "bqu"), ("brk", "brk"),
                    ("brq", "brq"), ("cos", "cosT"), ("sin", "sinT")):
        nc.sync.dma_start(t[nm][:], d[key][:])


def _consts_load(nc, t, d):
    nc.sync.dma_start(t["bvu_row"][:], d["bvu"][:])
    nc.gpsimd.partition_broadcast(t["bvu_bc"][:], t["bvu_row"][:])
    _consts_load_down(nc, t, d)
    _consts_load_rest(nc, t, d)


def _load_wo(nc, t, d):
    for hc in range(HPC):
        nc.sync.dma_start(t["wo"][hc][:], d["woT"][hc * 128:(hc + 1) * 128, :])


def _phaseA_pools(tc, pa):
    p = {}
    p["xa"] = pa.enter_context(tc.tile_pool(name="xa", bufs=2))
    p["kvq"] = pa.enter_context(tc.tile_pool(name="kvq", bufs=1))
    p["rp"] = pa.enter_context(tc.tile_pool(name="rp", bufs=2))
    p["psA"] = pa.enter_context(tc.tile_pool(name="psA", bufs=8,
                                             space="PSUM"))
    return p


def _load_x_chunk(nc, d, p, sc, eng=None):
    import concourse.mybir as mybir
    BF16 = mybir.dt.bfloat16
    ssl = slice(sc * SW, (sc + 1) * SW)
    xt = p["xa"].tile([128, NE * SW], BF16, name="xt")
    (eng or nc.sync).dma_start(
        xt[:].rearrange("p (e s) -> p e s", e=NE),
        d["xT"][:, ssl].rearrange("(e p) s -> p e s", p=128))
    return xt


def _emit_A(nc, tc, d, t, p, xts_pre=()):
    import concourse.mybir as mybir
    from concourse.alu_op_type import AluOpType
    F32 = mybir.dt.float32
    BF16 = mybir.dt.bfloat16
    K_t, Q_t, V_t, upw, dnw = t["K"], t["Q"], t["V"], t["upw"], t["dnw"]
    swap_mask = [i ^ 1 for i in range(32)]

    for sc in range(NSC):
        ssl = slice(sc * SW, (sc + 1) * SW)
        xt = (xts_pre[sc] if sc < len(xts_pre)
              else _load_x_chunk(nc, d, p, sc))
        cos_s = t["cos"][:, ssl]
        sin_s = t["sin"][:, ssl]

        def down_mm(wt, m):
            ps = p["psA"].tile([128, SW], F32, name="psA_t")
            for e in range(NE):
                nc.tensor.matmul(ps[:], wt[m][:, e * 128:(e + 1) * 128],
                                 xt[:, e * SW:(e + 1) * SW],
                                 start=(e == 0), stop=(e == NE - 1))
            return ps

        def rope(ps, bias_t, m, dst):
            # ps: [128 rows = 2 heads x 64 rope rows, SW]
            xb = p["rp"].tile([128, SW], BF16, name="xb")
            nc.scalar.add(xb[:], ps[:], bias_t[:, m:m + 1])
            sh = p["rp"].tile([128, SW], BF16, name="sh")
            nc.vector.stream_shuffle(sh[:], xb[:], swap_mask)
            t1 = p["rp"].tile([128, SW], BF16, name="t1")
            nc.vector.tensor_tensor(t1[:], xb[:], cos_s,
                                    op=AluOpType.mult)
            t2 = p["rp"].tile([128, SW], BF16, name="t2")
            nc.vector.tensor_tensor(t2[:], sh[:], sin_s,
                                    op=AluOpType.mult)
            nc.vector.tensor_tensor(dst[2 * m][R:D, ssl], t1[0:R, :],
                                    t2[0:R, :], op=AluOpType.add)
            nc.vector.tensor_tensor(dst[2 * m + 1][R:D, ssl], t1[R:D, :],
                                    t2[R:D, :], op=AluOpType.add)

        def up_mm(src, w, m):
            ps = p["psA"].tile([128, SW], F32, name="psA_t")
            for l in range(NL):
                nc.tensor.matmul(ps[:], w[l][:, m * 128:(m + 1) * 128],
                                 src[l][:], start=(l == 0),
                                 stop=(l == NL - 1))
            return ps

        # latent kv_d down-projection (replicated in batch group)
        kv_s = []
        for m in range(NL):
            ps = down_mm(dnw["kvd"], m)
            tl = p["kvq"].tile([128, SW], BF16, name=f"lat{m}")
            nc.scalar.add(tl[:], ps[:], t["bkvd"][:, m:m + 1])
            kv_s.append(tl)
        for m in range(2):  # k1 -> K rows 0..63
            ps = up_mm(kv_s, upw["ku"], m)
            nc.scalar.add(K_t[2 * m][0:R, ssl], ps[0:R, :],
                          t["bku"][0:R, m:m + 1])
            nc.scalar.add(K_t[2 * m + 1][0:R, ssl], ps[R:D, :],
                          t["bku"][R:D, m:m + 1])
        for j in range(SW // 128):  # V, (s, feat) layout
            ps = p["psA"].tile([128, HPC * D], F32, name="psA_t")
            for l in range(NL):
                nc.tensor.matmul(ps[:], kv_s[l][:, j * 128:(j + 1) * 128],
                                 upw["vu"][l][:], start=(l == 0),
                                 stop=(l == NL - 1))
            nc.vector.tensor_tensor(V_t[sc * (SW // 128) + j][:], ps[:],
                                    t["bvu_bc"][:], op=AluOpType.add)

        # latent q_d down-projection (slots shared with kv_s)
        q_s = []
        for m in range(NL):
            ps = down_mm(dnw["qd"], m)
            tl = p["kvq"].tile([128, SW], BF16, name=f"lat{m}")
            nc.scalar.add(tl[:], ps[:], t["bqd"][:, m:m + 1])
            q_s.append(tl)
        for m in range(2):  # q1 -> Q rows 0..63
            ps = up_mm(q_s, upw["qu"], m)
            nc.scalar.add(Q_t[2 * m][0:R, ssl], ps[0:R, :],
                          t["bqu"][0:R, m:m + 1])
            nc.scalar.add(Q_t[2 * m + 1][0:R, ssl], ps[R:D, :],
                          t["bqu"][R:D, m:m + 1])
        for m in range(2):  # rope-q from q_d
            ps = up_mm(q_s, upw["rq"], m)
            rope(ps, t["brq"], m, Q_t)
        # rope-k from x
        for m in range(2):
            ps = down_mm(dnw["rk"], m)
            rope(ps, t["brk"], m, K_t)


def _phaseB_pools(tc, pb):
    p = {}
    p["pe"] = pb.enter_context(tc.tile_pool(name="pe", bufs=3))
    p["ac"] = pb.enter_context(tc.tile_pool(name="ac", bufs=2))
    p["sm"] = pb.enter_context(tc.tile_pool(name="sm", bufs=2))
    p["cb"] = pb.enter_context(tc.tile_pool(name="cb", bufs=2))
    p["psS"] = pb.enter_context(tc.tile_pool(name="psS", bufs=2,
                                             space="PSUM"))
    p["psO"] = pb.enter_context(tc.tile_pool(name="psO", bufs=1,
                                             space="PSUM"))
    p["psR"] = pb.enter_context(tc.tile_pool(name="psR", bufs=2,
                                             space="PSUM"))
    return p


def _emit_B(nc, tc, d, t, p, att_t, mode="full"):
    import concourse.mybir as mybir
    from concourse.alu_op_type import AluOpType
    F32 = mybir.dt.float32
    F32R = mybir.dt.float32r
    BF16 = mybir.dt.bfloat16
    AF = mybir.ActivationFunctionType
    K_t, Q_t, V_t = t["K"], t["Q"], t["V"]

    LAG = 3  # PV trails QK/exp by LAG k-chunks so PE never waits on ACT

    for h in range(HPC):
        for qp in range(2):
            qa = slice((2 * qp) * 512, (2 * qp + 1) * 512)
            qb = slice((2 * qp + 1) * 512, (2 * qp + 2) * 512)
            oA = p["psO"].tile([128, 512], F32, name="oA")
            oB = p["psO"].tile([128, 512], F32, name="oB")
            # two interleaved bf16 row-sum chains on DVE (all-16bit ops
            # run at 2x rate, so they keep up with the exp cadence); the
            # fp32 combine for the ones-matmul happens once at the end
            acc0 = p["ac"].tile([128, 1024], BF16, name="acc0")
            acc1 = p["ac"].tile([128, 1024], BF16, name="acc1")
            accf = p["ac"].tile([128, 1024], F32R, name="accf")
            accs = (acc0, acc1)
            pes = {}

            def pv(kk):
                pe = pes.pop(kk)
                nc.tensor.matmul(oA[:], V_t[kk][:, h * D:(h + 1) * D],
                                 pe[:, 0:512], start=(kk == 0),
                                 stop=(kk == NKC - 1))
                nc.tensor.matmul(oB[:], V_t[kk][:, h * D:(h + 1) * D],
                                 pe[:, 512:1024], start=(kk == 0),
                                 stop=(kk == NKC - 1))

            for kk in range(NKC):
                ksl = slice(kk * 128, (kk + 1) * 128)
                pp = p["psS"].tile([128, 1024], F32, name="pp")
                nc.tensor.matmul(pp[:, 0:512], K_t[h][:, ksl], Q_t[h][:, qa],
                                 start=True, stop=True)
                nc.tensor.matmul(pp[:, 512:1024], K_t[h][:, ksl],
                                 Q_t[h][:, qb], start=True, stop=True)
                if mode == "qk":
                    continue
                pe = p["pe"].tile([128, 1024], BF16, name="pet", bufs=5)
                nc.scalar.activation(pe[:], pp[:], AF.Exp, scale=SCALE)
                if mode == "qke":
                    continue
                # row-sum accumulation (keys land on partitions later)
                acc = accs[kk % 2]
                if kk < 2:
                    nc.vector.tensor_copy(acc[:], pe[:])
                else:
                    nc.vector.tensor_tensor(acc[:], pe[:], acc[:],
                                            op=AluOpType.add)
                pes[kk] = pe
                if kk >= LAG:
                    pv(kk - LAG)
            if mode != "full":
                continue
            for kk in range(NKC - LAG, NKC):
                pv(kk)
            # r[q] = sum_p acc[p, q] via ones-matmul; then 1/r broadcast
            nc.vector.tensor_tensor(accf[:], acc0[:], acc1[:],
                                    op=AluOpType.add)
            # sums live in their own PSUM pool (NOT the pp pool): when
            # they shared pp's slots, the next iteration's QK matmuls
            # waited on the reciprocal to drain the slot, starving the
            # Activation engine ~3.5us per head-block.
            ci = p["sm"].tile([1, 1024], F32, name="ci")
            for half in range(2):
                hs = slice(half * 512, (half + 1) * 512)
                sums = p["psR"].tile([1, 512], F32, name="sums")
                nc.tensor.matmul(sums[:], t["ones"][:], accf[:, hs],
                                 start=True, stop=True)
                nc.vector.reciprocal_approx_fast(ci[:, hs], sums[:])
            cb = p["cb"].tile([128, 1024], F32, name="cbt")
            nc.gpsimd.partition_broadcast(cb[:], ci[:])
            nc.vector.tensor_tensor(att_t[h][:, qa], oA[:], cb[:, 0:512],
                                    op=AluOpType.mult)
            nc.vector.tensor_tensor(att_t[h][:, qb], oB[:], cb[:, 512:1024],
                                    op=AluOpType.mult)


def _phaseC_pools(tc, pc):
    p = {}
    p["oc"] = pc.enter_context(tc.tile_pool(name="oc", bufs=3))
    p["psC"] = pc.enter_context(tc.tile_pool(name="psC", bufs=4,
                                             space="PSUM"))
    return p


def _emit_C(nc, tc, d, t, p, att_t):
    import concourse.mybir as mybir
    F32 = mybir.dt.float32
    BF16 = mybir.dt.bfloat16
    wo_t = t["wo"]

    for sj in range(S // 128):
        for ocn in range(E // 512):
            ps = p["psC"].tile([128, 512], F32, name="psC_t")
            for hc in range(HPC):
                nc.tensor.matmul(ps[:], att_t[hc][:, sj * 128:(sj + 1) * 128],
                                 wo_t[hc][:, ocn * 512:(ocn + 1) * 512],
                                 start=(hc == 0), stop=(hc == HPC - 1))
            ob = p["oc"].tile([128, 512], BF16, name="ob")
            nc.vector.tensor_copy(ob[:], ps[:])
            # alternate DMA issue queues: a single queue's ~850ns issue
            # cost per tile would gate the drain of 64 output tiles
            eng = nc.sync if ocn % 2 == 0 else nc.scalar
            eng.dma_start(
                d["out"][sj * 128:(sj + 1) * 128,
                         ocn * 512:(ocn + 1) * 512], ob[:])


def _build_program(loop=None):
    """loop=None: normal kernel. loop=(phase, n): benchmark variant with a
    hardware For_i loop repeating one phase n times."""
    import concourse.bacc as bacc
    import concourse.mybir as mybir
    import concourse.tile as tile

    BF16 = mybir.dt.bfloat16

    nc = bacc.Bacc("TRN2", target_bir_lowering=False, debug=False,
                   num_devices=NCORES)
    d = _mk(nc)

    with tile.TileContext(nc) as tc, ExitStack() as top:
        if loop is None:
            # Allocate persistent pools first (pool release is LIFO), and
            # stagger the preload DMA queue by first-use so nothing the
            # first ~50us of compute needs sits behind 10MB of weights:
            # x0, down-weights, x1, everything else.  x2/x3 prefetch on
            # the (otherwise idle in phase A) gpsimd queue, where their
            # wait for an xa slot blocks nothing.
            t = _consts(nc, tc, top, d, loads=False)
            with ExitStack() as pa:
                pA = _phaseA_pools(tc, pa)
                xts_pre = [_load_x_chunk(nc, d, pA, 0)]
                nc.sync.dma_start(t["bvu_row"][:], d["bvu"][:])
                nc.gpsimd.partition_broadcast(t["bvu_bc"][:],
                                              t["bvu_row"][:])
                _consts_load_down(nc, t, d)
                xts_pre.append(_load_x_chunk(nc, d, pA, 1))
                _consts_load_rest(nc, t, d)
                xts_pre.append(_load_x_chunk(nc, d, pA, 2, eng=nc.gpsimd))
                xts_pre.append(_load_x_chunk(nc, d, pA, 3, eng=nc.gpsimd))
                _emit_A(nc, tc, d, t, pA, xts_pre)
            _load_wo(nc, t, d)  # hide the Wo load under phase B
            with ExitStack() as pb:
                att_pool = pb.enter_context(tc.tile_pool(name="att", bufs=1))
                att_t = [att_pool.tile([128, S], BF16, name=f"att{h}")
                         for h in range(HPC)]
                with ExitStack() as pbi:
                    pB = _phaseB_pools(tc, pbi)
                    _emit_B(nc, tc, d, t, pB, att_t)
                with ExitStack() as pc:
                    pC = _phaseC_pools(tc, pc)
                    _emit_C(nc, tc, d, t, pC, att_t)
        else:
            phase, n = loop
            t = _consts(nc, tc, top, d)

            def _fill(tile_, w):
                nc.sync.dma_start(tile_[:], d["xT"][0:128, 0:w])

            with ExitStack() as ps_:
                if phase == "A":
                    pA = _phaseA_pools(tc, ps_)
                    with tc.For_i(0, n, 1):
                        _emit_A(nc, tc, d, t, pA)
                elif phase.startswith("B"):
                    mode = {"B": "full", "B0": "qk", "B1": "qke"}[phase]
                    for h in range(HPC):
                        _fill(t["K"][h], S)
                        _fill(t["Q"][h], S)
                    for i in range(NKC):
                        _fill(t["V"][i], HPC * D)
                    att_pool = ps_.enter_context(
                        tc.tile_pool(name="att", bufs=1))
                    att_t = [att_pool.tile([128, S], BF16, name=f"att{h}")
                             for h in range(HPC)]
                    pB = _phaseB_pools(tc, ps_)
                    with tc.For_i(0, n, 1):
                        _emit_B(nc, tc, d, t, pB, att_t, mode)
                elif phase == "C":
                    att_pool = ps_.enter_context(
                        tc.tile_pool(name="att", bufs=1))
                    att_t = [att_pool.tile([128, S], BF16, name=f"att{h}")
                             for h in range(HPC)]
                    for h in range(HPC):
                        _fill(att_t[h], S)
                    _load_wo(nc, t, d)
                    pC = _phaseC_pools(tc, ps_)
                    with tc.For_i(0, n, 1):
                        _emit_C(nc, tc, d, t, pC, att_t)
                else:
                    raise ValueError(phase)

    nc.compile()
    return nc


def _bf16(a):
    import ml_dtypes
    return np.ascontiguousarray(np.asarray(a, dtype=np.float32)).astype(
        ml_dtypes.bfloat16)


def _rope_tables():
    inv_freq = 1.0 / (10000.0 ** (np.arange(0, R, 2, dtype=np.float64) / R))
    t = np.arange(S, dtype=np.float64)
    freqs = np.outer(t, inv_freq)                       # (S, R/2)
    emb = np.concatenate([freqs, freqs], axis=-1)       # (S, R)
    cos = np.cos(emb).astype(np.float32)                # (S, R)
    sin = np.sin(emb).astype(np.float32)
    perm = np.array([(j // 2) if j % 2 == 0 else (j // 2) + R // 2
                     for j in range(R)])
    sign = np.array([-1.0 if j % 2 == 0 else 1.0
                     for j in range(R)], dtype=np.float32)
    cos_p = cos[:, perm].T.copy()                       # (R, S)
    sin_p = (sin[:, perm] * sign[None, :]).T.copy()     # (R, S)
    cosT = np.concatenate([cos_p, cos_p], axis=0)       # (128, S)
    sinT = np.concatenate([sin_p, sin_p], axis=0)
    return cosT, sinT, perm


def _per_core_inputs(inputs, core):
    b, hg = divmod(core, HPC)
    cosT, sinT, perm = _rope_tables()
    hsl64 = np.concatenate([hg * HPC * R + h * R + perm
                            for h in range(HPC)])       # permuted rope rows
    hs64 = slice(hg * HPC * R, (hg + 1) * HPC * R)      # natural 64-rows
    hs128 = slice(hg * HPC * D, (hg + 1) * HPC * D)     # natural 128-rows

    x = np.asarray(inputs["x"], dtype=np.float32)
    f = np.float32
    im = {
        "xT": _bf16(x[b].T),
        "wkvdT": _bf16(np.asarray(inputs["Wkv_d"], f).T),
        "wqdT": _bf16(np.asarray(inputs["Wq_d"], f).T),
        "wrkT": _bf16(np.asarray(inputs["Wrk"], f)[hsl64].T),
        "wkuT": _bf16(np.asarray(inputs["Wk_u"], f)[hs64].T),
        "wquT": _bf16(np.asarray(inputs["Wq_u"], f)[hs64].T),
        "wrqT": _bf16(np.asarray(inputs["Wrq"], f)[hsl64].T),
        "wvuT": _bf16(np.asarray(inputs["Wv_u"], f)[hs128].T),
        "woT": _bf16(np.asarray(inputs["Wo"], f).T[hs128]),
        "bkvd": np.ascontiguousarray(
            np.asarray(inputs["bkv_d"], f).reshape(NL, 128).T),
        "bqd": np.ascontiguousarray(
            np.asarray(inputs["bq_d"], f).reshape(NL, 128).T),
        "bku": np.ascontiguousarray(
            np.asarray(inputs["bk_u"], f)[hs64].reshape(2, 128).T),
        "bqu": np.ascontiguousarray(
            np.asarray(inputs["bq_u"], f)[hs64].reshape(2, 128).T),
        "brk": np.ascontiguousarray(
            np.asarray(inputs["brk"], f)[hsl64].reshape(2, 128).T),
        "brq": np.ascontiguousarray(
            np.asarray(inputs["brq"], f)[hsl64].reshape(2, 128).T),
        "bvu": np.ascontiguousarray(
            np.asarray(inputs["bv_u"], f)[hs128].reshape(1, HPC * D)),
        "onesd": np.ones((128, 1), dtype=np.float32),
        "cosT": _bf16(cosT),
        "sinT": _bf16(sinT),
    }
    return im


def _get_runtime(loop=None):
    key = loop
    if key in _RT:
        return _RT[key]
    import jax
    import numpy as _np
    from jax.sharding import Mesh, PartitionSpec
    from jax.experimental.shard_map import shard_map

    import concourse.mybir as mybir
    from concourse import bass2jax

    nc = _build_program(loop)
    bass2jax.install_neuronx_cc_hook()

    partition_name = (nc.partition_id_tensor.name
                      if nc.partition_id_tensor else None)
    in_names, out_names, out_avals, zero_shapes = [], [], [], []
    for alloc in nc.m.functions[0].allocations:
        if not isinstance(alloc, mybir.MemoryLocationSet):
            continue
        name = alloc.memorylocations[0].name
        if alloc.kind == "ExternalInput":
            if name != partition_name:
                in_names.append(name)
        elif alloc.kind == "ExternalOutput":
            out_names.append(name)
            np_dt = mybir.dt.np(alloc.dtype)
            out_avals.append(jax.core.ShapedArray(
                tuple(alloc.tensor_shape), np_dt))
            zero_shapes.append((tuple(alloc.tensor_shape), np_dt))

    n_params = len(in_names)
    n_outs = len(out_names)
    all_in_names = list(in_names) + list(out_names)
    if partition_name is not None:
        all_in_names.append(partition_name)

    def _body(*args):
        operands = list(args)
        if partition_name is not None:
            operands.append(bass2jax.partition_id_tensor())
        outs = bass2jax._bass_exec_p.bind(
            *operands,
            out_avals=tuple(out_avals),
            in_names=tuple(all_in_names),
            out_names=tuple(out_names),
            lowering_input_output_aliases=(),
            sim_require_finite=True,
            sim_require_nnan=True,
            nc=nc,
        )
        return tuple(outs)

    devices = jax.devices()[:NCORES]
    mesh = Mesh(_np.asarray(devices), ("core",))
    in_specs = (PartitionSpec("core"),) * (n_params + n_outs)
    out_specs = (PartitionSpec("core"),) * n_outs
    donate = tuple(range(n_params, n_params + n_outs))
    sharded = jax.jit(
        shard_map(_body, mesh=mesh, in_specs=in_specs, out_specs=out_specs,
                  check_rep=False),
        donate_argnums=donate, keep_unused=True)

    _RT[key] = dict(sharded=sharded, in_names=in_names, out_names=out_names,
                    zero_shapes=zero_shapes, n_outs=n_outs, nc=nc)
    return _RT[key]


def _run_cores(in_maps):
    rt = _get_runtime()
    import numpy as _np
    concat_in = [
        _np.concatenate([in_maps[c][name] for c in range(NCORES)], axis=0)
        for name in rt["in_names"]
    ]
    concat_zeros = [
        _np.zeros((NCORES * shp[0],) + shp[1:], dt)
        for (shp, dt) in rt["zero_shapes"]
    ]
    out_arrs = rt["sharded"](*concat_in, *concat_zeros)
    res = []
    for c in range(NCORES):
        m = {}
        for i, name in enumerate(rt["out_names"]):
            shp, dt = rt["zero_shapes"][i]
            m[name] = _np.asarray(out_arrs[i]).reshape((NCORES,) + shp)[c]
        res.append(m)
    return res


def kernel(**inputs):
    in_maps = [_per_core_inputs(inputs, c) for c in range(NCORES)]
    res = _run_cores(in_maps)
    bo = np.asarray(inputs["bo"], dtype=np.float32)
    final = np.empty((B, S, E), dtype=np.float32)
    for b in range(B):
        acc = res[HPC * b]["out"].astype(np.float32).copy()
        for g in range(1, HPC):
            acc += res[HPC * b + g]["out"]
        final[b] = acc + bo[None, :]
    return final


# revision 26
# speedup vs baseline: 1.0616x; 1.0308x over previous
"""MultiHeadLatentAttention TRN2 kernel (v2, bf16).

Sharding: 8 cores = 2 (batch) x 4 (head groups of 4 heads).
Each core computes, for its batch b and heads hg*4..hg*4+3:
  - latent down-projections kv_d, q_d (replicated within the batch group)
  - per-head up-projections K^T, Q^T (with RoPE), V
  - full attention for its 4 heads
  - partial output projection (its 512 columns of Wo's input dim)
Partial outputs are summed on the host (+ bo).

Optimizations vs the first working version (697us/core -> 463us/core on
HW, NTFF-profiled):
  - all matmul operands in bf16 (tolerance test: full-pipeline max-rel
    error ~3e-3 vs the 2e-2 gate); halves LDWEIGHTS + SBUF + DMA.
  - all weights + cos/sin resident in SBUF, loaded once (v1 reloaded
    down-proj weights every s-chunk: ~30MB of redundant DMA); x chunks
    double-buffered and prefetched ahead of the weight preloads so the
    first matmul starts early.
  - softmax row-sum accumulation as two interleaved bf16 chains on DVE
    (all-16-bit DVE ops run at 2x; the Pool engine measured ~2.5ns/elem
    and can't read PSUM, so it only does the 1/sum partition-broadcast);
    1/sum via reciprocal_approx_fast; PSUM double-buffered so the
    softmax epilogue never blocks the next head's QK matmuls.
  - output written in bf16 (halves the HBM write traffic that stalled
    the phase-C PSUM drain).
Phase floors per core: A (projections) ~175us PE-bound, B (attention)
~190us ACT-bound (16.8M exp elements at 0.833ns/elem), C (out-proj)
~65us PE-bound.
"""

import sys

sys.path.insert(0, "/opt/trn_rl_repo")

from contextlib import ExitStack

import numpy as np

H = 16
E = 2048
LAT = E // 4          # 512
D = E // H            # 128
R = D // 2            # 64
B, S = 2, 2048
HPC = H // 4          # 4 heads per core
NCORES = 8
NE = E // 128         # 16 contraction chunks over E
NL = LAT // 128       # 4 contraction chunks over LAT
SW = 512              # s-chunk width for projections
NSC = S // SW         # 4 s-chunks
NKC = S // 128        # 16 key chunks
SCALE = 1.0 / float(np.sqrt(D))

_RT = {}  # cached runtimes


def _mk(nc):
    """Declare DRAM I/O; returns dict of handles."""
    import concourse.mybir as mybir
    F32 = mybir.dt.float32
    F32R = mybir.dt.float32r
    BF16 = mybir.dt.bfloat16
    d = {}
    d["xT"] = nc.dram_tensor("xT", [E, S], BF16, kind="ExternalInput")
    d["wkvdT"] = nc.dram_tensor("wkvdT", [E, LAT], BF16, kind="ExternalInput")
    d["wqdT"] = nc.dram_tensor("wqdT", [E, LAT], BF16, kind="ExternalInput")
    d["wrkT"] = nc.dram_tensor("wrkT", [E, HPC * R], BF16,
                               kind="ExternalInput")
    d["wkuT"] = nc.dram_tensor("wkuT", [LAT, HPC * R], BF16,
                               kind="ExternalInput")
    d["wquT"] = nc.dram_tensor("wquT", [LAT, HPC * R], BF16,
                               kind="ExternalInput")
    d["wrqT"] = nc.dram_tensor("wrqT", [LAT, HPC * R], BF16,
                               kind="ExternalInput")
    d["wvuT"] = nc.dram_tensor("wvuT", [LAT, HPC * D], BF16,
                               kind="ExternalInput")
    d["woT"] = nc.dram_tensor("woT", [HPC * D, E], BF16,
                              kind="ExternalInput")
    d["bkvd"] = nc.dram_tensor("bkvd", [128, NL], F32, kind="ExternalInput")
    d["bqd"] = nc.dram_tensor("bqd", [128, NL], F32, kind="ExternalInput")
    d["bku"] = nc.dram_tensor("bku", [128, 2], F32, kind="ExternalInput")
    d["bqu"] = nc.dram_tensor("bqu", [128, 2], F32, kind="ExternalInput")
    d["brk"] = nc.dram_tensor("brk", [128, 2], F32, kind="ExternalInput")
    d["brq"] = nc.dram_tensor("brq", [128, 2], F32, kind="ExternalInput")
    d["bvu"] = nc.dram_tensor("bvu", [1, HPC * D], F32, kind="ExternalInput")
    d["onesd"] = nc.dram_tensor("onesd", [128, 1], F32R,
                                kind="ExternalInput")
    d["cosT"] = nc.dram_tensor("cosT", [128, S], BF16, kind="ExternalInput")
    d["sinT"] = nc.dram_tensor("sinT", [128, S], BF16, kind="ExternalInput")
    d["out"] = nc.dram_tensor("out", [S, E], BF16, kind="ExternalOutput")
    return d


def _consts(nc, tc, top, d, loads=True):
    """Persistent tiles: K/Q/V storage, biases, ones, all weights.
    With loads=False only allocates; call _consts_load to emit DMAs."""
    import concourse.mybir as mybir
    F32 = mybir.dt.float32
    F32R = mybir.dt.float32r
    BF16 = mybir.dt.bfloat16

    kq_pool = top.enter_context(tc.tile_pool(name="kq", bufs=1))
    v_pool = top.enter_context(tc.tile_pool(name="vp", bufs=1))
    cpool = top.enter_context(tc.tile_pool(name="cp", bufs=1))

    t = {}
    t["K"] = [kq_pool.tile([128, S], BF16, name=f"Kt{h}") for h in range(HPC)]
    t["Q"] = [kq_pool.tile([128, S], BF16, name=f"Qt{h}") for h in range(HPC)]
    t["V"] = [v_pool.tile([128, HPC * D], BF16, name=f"Vt{i}")
              for i in range(NKC)]

    def al(name, shape, dt=F32):
        return cpool.tile(shape, dt, name=name)

    # Down-projection weights: per output m-chunk, a [128, NE*128] tile
    # whose e-th column block is the lhsT for contraction chunk e.
    t["dnw"] = {
        nm: [al(f"wd_{nm}{m}", [128, NE * 128], BF16)
             for m in range(nm_chunks)]
        for nm, nm_chunks in (("kvd", NL), ("qd", NL), ("rk", 2))
    }
    t["upw"] = {
        nm: [al(f"w{nm}{l}", [128, w], BF16) for l in range(NL)]
        for nm, w in (("ku", HPC * R), ("qu", HPC * R),
                      ("rq", HPC * R), ("vu", HPC * D))
    }
    t["ones"] = al("ones_t", [128, 1], F32R)
    t["bkvd"] = al("bkvd_t", [128, NL])
    t["bqd"] = al("bqd_t", [128, NL])
    t["bku"] = al("bku_t", [128, 2])
    t["bqu"] = al("bqu_t", [128, 2])
    t["brk"] = al("brk_t", [128, 2])
    t["brq"] = al("brq_t", [128, 2])
    t["cos"] = al("cos_t", [128, S], BF16)
    t["sin"] = al("sin_t", [128, S], BF16)
    t["bvu_row"] = al("bvu_row", [1, HPC * D])
    t["bvu_bc"] = al("bvu_bc", [128, HPC * D])
    # wo loads are issued later (before phase C) via _load_wo
    t["wo"] = [al(f"wo{hc}", [128, E], BF16) for hc in range(HPC)]
    if loads:
        _consts_load(nc, t, d)
    return t


def _consts_load_down(nc, t, d):
    """Down-projection weight DMAs (phase A's matmuls need these first)."""
    dmap = {"kvd": "wkvdT", "qd": "wqdT", "rk": "wrkT"}
    for nm in ("kvd", "qd", "rk"):
        tl = t["dnw"][nm]
        for m in range(len(tl)):
            nc.sync.dma_start(
                tl[m][:].rearrange("p (e c) -> p e c", e=NE),
                d[dmap[nm]][:, m * 128:(m + 1) * 128].rearrange(
                    "(e p) c -> p e c", p=128))


def _consts_load_rest(nc, t, d):
    umap = {"ku": "wkuT", "qu": "wquT", "rq": "wrqT", "vu": "wvuT"}
    for nm in ("ku", "qu", "rq", "vu"):
        tl = t["upw"][nm]
        for l in range(NL):
            nc.sync.dma_start(tl[l][:], d[umap[nm]][l * 128:(l + 1) * 128, :])
    for nm, key in (("ones", "onesd"), ("bkvd", "bkvd"), ("bqd", "bqd"),
                    ("bku", "bku"), ("bqu", "bqu"), ("brk", "brk"),
                    ("brq", "brq"), ("cos", "cosT"), ("sin", "sinT"),
                    ("bvu_row", "bvu")):
        nc.sync.dma_start(t[nm][:], d[key][:])
    nc.gpsimd.partition_broadcast(t["bvu_bc"][:], t["bvu_row"][:])


def _consts_load(nc, t, d):
    _consts_load_down(nc, t, d)
    _consts_load_rest(nc, t, d)


def _load_wo(nc, t, d):
    for hc in range(HPC):
        nc.sync.dma_start(t["wo"][hc][:], d["woT"][hc * 128:(hc + 1) * 128, :])


def _phaseA_pools(tc, pa):
    p = {}
    p["xa"] = pa.enter_context(tc.tile_pool(name="xa", bufs=2))
    p["kvq"] = pa.enter_context(tc.tile_pool(name="kvq", bufs=1))
    p["rp"] = pa.enter_context(tc.tile_pool(name="rp", bufs=2))
    p["psA"] = pa.enter_context(tc.tile_pool(name="psA", bufs=6,
                                             space="PSUM"))
    return p


def _load_x_chunk(nc, d, p, sc, eng=None):
    import concourse.mybir as mybir
    BF16 = mybir.dt.bfloat16
    ssl = slice(sc * SW, (sc + 1) * SW)
    xt = p["xa"].tile([128, NE * SW], BF16, name="xt")
    (eng or nc.sync).dma_start(
        xt[:].rearrange("p (e s) -> p e s", e=NE),
        d["xT"][:, ssl].rearrange("(e p) s -> p e s", p=128))
    return xt


def _emit_A(nc, tc, d, t, p, xts_pre=()):
    import concourse.mybir as mybir
    from concourse.alu_op_type import AluOpType
    F32 = mybir.dt.float32
    BF16 = mybir.dt.bfloat16
    K_t, Q_t, V_t, upw, dnw = t["K"], t["Q"], t["V"], t["upw"], t["dnw"]
    swap_mask = [i ^ 1 for i in range(32)]

    for sc in range(NSC):
        ssl = slice(sc * SW, (sc + 1) * SW)
        xt = (xts_pre[sc] if sc < len(xts_pre)
              else _load_x_chunk(nc, d, p, sc))
        cos_s = t["cos"][:, ssl]
        sin_s = t["sin"][:, ssl]

        def down_mm(wt, m):
            ps = p["psA"].tile([128, SW], F32, name="psA_t")
            for e in range(NE):
                nc.tensor.matmul(ps[:], wt[m][:, e * 128:(e + 1) * 128],
                                 xt[:, e * SW:(e + 1) * SW],
                                 start=(e == 0), stop=(e == NE - 1))
            return ps

        def rope(ps, bias_t, m, dst):
            # ps: [128 rows = 2 heads x 64 rope rows, SW]
            xb = p["rp"].tile([128, SW], BF16, name="xb")
            nc.scalar.add(xb[:], ps[:], bias_t[:, m:m + 1])
            sh = p["rp"].tile([128, SW], BF16, name="sh")
            nc.vector.stream_shuffle(sh[:], xb[:], swap_mask)
            t1 = p["rp"].tile([128, SW], BF16, name="t1")
            nc.vector.tensor_tensor(t1[:], xb[:], cos_s,
                                    op=AluOpType.mult)
            t2 = p["rp"].tile([128, SW], BF16, name="t2")
            nc.vector.tensor_tensor(t2[:], sh[:], sin_s,
                                    op=AluOpType.mult)
            nc.vector.tensor_tensor(dst[2 * m][R:D, ssl], t1[0:R, :],
                                    t2[0:R, :], op=AluOpType.add)
            nc.vector.tensor_tensor(dst[2 * m + 1][R:D, ssl], t1[R:D, :],
                                    t2[R:D, :], op=AluOpType.add)

        def up_mm(src, w, m):
            ps = p["psA"].tile([128, SW], F32, name="psA_t")
            for l in range(NL):
                nc.tensor.matmul(ps[:], w[l][:, m * 128:(m + 1) * 128],
                                 src[l][:], start=(l == 0),
                                 stop=(l == NL - 1))
            return ps

        # latent kv_d down-projection (replicated in batch group)
        kv_s = []
        for m in range(NL):
            ps = down_mm(dnw["kvd"], m)
            tl = p["kvq"].tile([128, SW], BF16, name=f"lat{m}")
            nc.scalar.add(tl[:], ps[:], t["bkvd"][:, m:m + 1])
            kv_s.append(tl)
        for m in range(2):  # k1 -> K rows 0..63
            ps = up_mm(kv_s, upw["ku"], m)
            nc.scalar.add(K_t[2 * m][0:R, ssl], ps[0:R, :],
                          t["bku"][0:R, m:m + 1])
            nc.scalar.add(K_t[2 * m + 1][0:R, ssl], ps[R:D, :],
                          t["bku"][R:D, m:m + 1])
        for j in range(SW // 128):  # V, (s, feat) layout
            ps = p["psA"].tile([128, HPC * D], F32, name="psA_t")
            for l in range(NL):
                nc.tensor.matmul(ps[:], kv_s[l][:, j * 128:(j + 1) * 128],
                                 upw["vu"][l][:], start=(l == 0),
                                 stop=(l == NL - 1))
            nc.vector.tensor_tensor(V_t[sc * (SW // 128) + j][:], ps[:],
                                    t["bvu_bc"][:], op=AluOpType.add)

        # latent q_d down-projection (slots shared with kv_s)
        q_s = []
        for m in range(NL):
            ps = down_mm(dnw["qd"], m)
            tl = p["kvq"].tile([128, SW], BF16, name=f"lat{m}")
            nc.scalar.add(tl[:], ps[:], t["bqd"][:, m:m + 1])
            q_s.append(tl)
        for m in range(2):  # q1 -> Q rows 0..63
            ps = up_mm(q_s, upw["qu"], m)
            nc.scalar.add(Q_t[2 * m][0:R, ssl], ps[0:R, :],
                          t["bqu"][0:R, m:m + 1])
            nc.scalar.add(Q_t[2 * m + 1][0:R, ssl], ps[R:D, :],
                          t["bqu"][R:D, m:m + 1])
        for m in range(2):  # rope-q from q_d
            ps = up_mm(q_s, upw["rq"], m)
            rope(ps, t["brq"], m, Q_t)
        # rope-k from x
        for m in range(2):
            ps = down_mm(dnw["rk"], m)
            rope(ps, t["brk"], m, K_t)


def _phaseB_pools(tc, pb):
    p = {}
    p["pe"] = pb.enter_context(tc.tile_pool(name="pe", bufs=3))
    p["ac"] = pb.enter_context(tc.tile_pool(name="ac", bufs=2))
    p["sm"] = pb.enter_context(tc.tile_pool(name="sm", bufs=2))
    p["cb"] = pb.enter_context(tc.tile_pool(name="cb", bufs=2))
    p["psS"] = pb.enter_context(tc.tile_pool(name="psS", bufs=2,
                                             space="PSUM"))
    p["psO"] = pb.enter_context(tc.tile_pool(name="psO", bufs=1,
                                             space="PSUM"))
    p["psR"] = pb.enter_context(tc.tile_pool(name="psR", bufs=2,
                                             space="PSUM"))
    return p


def _emit_B(nc, tc, d, t, p, att_t, mode="full"):
    import concourse.mybir as mybir
    from concourse.alu_op_type import AluOpType
    F32 = mybir.dt.float32
    F32R = mybir.dt.float32r
    BF16 = mybir.dt.bfloat16
    AF = mybir.ActivationFunctionType
    K_t, Q_t, V_t = t["K"], t["Q"], t["V"]

    LAG = 3  # PV trails QK/exp by LAG k-chunks so PE never waits on ACT

    for h in range(HPC):
        for qp in range(2):
            qa = slice((2 * qp) * 512, (2 * qp + 1) * 512)
            qb = slice((2 * qp + 1) * 512, (2 * qp + 2) * 512)
            oA = p["psO"].tile([128, 512], F32, name="oA")
            oB = p["psO"].tile([128, 512], F32, name="oB")
            # two interleaved bf16 row-sum chains on DVE (all-16bit ops
            # run at 2x rate, so they keep up with the exp cadence); the
            # fp32 combine for the ones-matmul happens once at the end
            acc0 = p["ac"].tile([128, 1024], BF16, name="acc0")
            acc1 = p["ac"].tile([128, 1024], BF16, name="acc1")
            accf = p["ac"].tile([128, 1024], F32R, name="accf")
            accs = (acc0, acc1)
            pes = {}

            def pv(kk):
                pe = pes.pop(kk)
                nc.tensor.matmul(oA[:], V_t[kk][:, h * D:(h + 1) * D],
                                 pe[:, 0:512], start=(kk == 0),
                                 stop=(kk == NKC - 1))
                nc.tensor.matmul(oB[:], V_t[kk][:, h * D:(h + 1) * D],
                                 pe[:, 512:1024], start=(kk == 0),
                                 stop=(kk == NKC - 1))

            for kk in range(NKC):
                ksl = slice(kk * 128, (kk + 1) * 128)
                pp = p["psS"].tile([128, 1024], F32, name="pp")
                nc.tensor.matmul(pp[:, 0:512], K_t[h][:, ksl], Q_t[h][:, qa],
                                 start=True, stop=True)
                nc.tensor.matmul(pp[:, 512:1024], K_t[h][:, ksl],
                                 Q_t[h][:, qb], start=True, stop=True)
                if mode == "qk":
                    continue
                pe = p["pe"].tile([128, 1024], BF16, name="pet", bufs=5)
                nc.scalar.activation(pe[:], pp[:], AF.Exp, scale=SCALE)
                if mode == "qke":
                    continue
                # row-sum accumulation (keys land on partitions later)
                acc = accs[kk % 2]
                if kk < 2:
                    nc.vector.tensor_copy(acc[:], pe[:])
                else:
                    nc.vector.tensor_tensor(acc[:], pe[:], acc[:],
                                            op=AluOpType.add)
                pes[kk] = pe
                if kk >= LAG:
                    pv(kk - LAG)
            if mode != "full":
                continue
            for kk in range(NKC - LAG, NKC):
                pv(kk)
            # r[q] = sum_p acc[p, q] via ones-matmul; then 1/r broadcast
            nc.vector.tensor_tensor(accf[:], acc0[:], acc1[:],
                                    op=AluOpType.add)
            # sums live in their own PSUM pool (NOT the pp pool): when
            # they shared pp's slots, the next iteration's QK matmuls
            # waited on the reciprocal to drain the slot, starving the
            # Activation engine ~3.5us per head-block.
            ci = p["sm"].tile([1, 1024], F32, name="ci")
            for half in range(2):
                hs = slice(half * 512, (half + 1) * 512)
                sums = p["psR"].tile([1, 512], F32, name="sums")
                nc.tensor.matmul(sums[:], t["ones"][:], accf[:, hs],
                                 start=True, stop=True)
                nc.vector.reciprocal_approx_fast(ci[:, hs], sums[:])
            cb = p["cb"].tile([128, 1024], F32, name="cbt")
            nc.gpsimd.partition_broadcast(cb[:], ci[:])
            nc.vector.tensor_tensor(att_t[h][:, qa], oA[:], cb[:, 0:512],
                                    op=AluOpType.mult)
            nc.vector.tensor_tensor(att_t[h][:, qb], oB[:], cb[:, 512:1024],
                                    op=AluOpType.mult)


def _phaseC_pools(tc, pc):
    p = {}
    # deep buffering: each 128KB output DMA holds its ob staging tile
    # ~5.7us, so shallow pools throttle the phase-C drain behind the DMA
    # engines rather than the PE
    p["oc"] = pc.enter_context(tc.tile_pool(name="oc", bufs=8))
    p["psC"] = pc.enter_context(tc.tile_pool(name="psC", bufs=8,
                                             space="PSUM"))
    return p


def _emit_C(nc, tc, d, t, p, att_t):
    import concourse.mybir as mybir
    F32 = mybir.dt.float32
    BF16 = mybir.dt.bfloat16
    wo_t = t["wo"]

    for sj in range(S // 128):
        for ocn in range(E // 512):
            ps = p["psC"].tile([128, 512], F32, name="psC_t")
            for hc in range(HPC):
                nc.tensor.matmul(ps[:], att_t[hc][:, sj * 128:(sj + 1) * 128],
                                 wo_t[hc][:, ocn * 512:(ocn + 1) * 512],
                                 start=(hc == 0), stop=(hc == HPC - 1))
            ob = p["oc"].tile([128, 512], BF16, name="ob")
            nc.vector.tensor_copy(ob[:], ps[:])
            # alternate DMA issue queues: a single queue's ~850ns issue
            # cost per tile would gate the drain of 64 output tiles
            eng = nc.sync if ocn % 2 == 0 else nc.scalar
            eng.dma_start(
                d["out"][sj * 128:(sj + 1) * 128,
                         ocn * 512:(ocn + 1) * 512], ob[:])


def _build_program(loop=None):
    """loop=None: normal kernel. loop=(phase, n): benchmark variant with a
    hardware For_i loop repeating one phase n times."""
    import concourse.bacc as bacc
    import concourse.mybir as mybir
    import concourse.tile as tile

    BF16 = mybir.dt.bfloat16

    nc = bacc.Bacc("TRN2", target_bir_lowering=False, debug=False,
                   num_devices=NCORES)
    d = _mk(nc)

    with tile.TileContext(nc) as tc, ExitStack() as top:
        if loop is None:
            # Allocate persistent pools first (pool release is LIFO), and
            # stagger the preload DMA queue by first-use so nothing the
            # first ~50us of compute needs sits behind 10MB of weights:
            # x0, down-weights, x1, everything else.  x2/x3 prefetch on
            # the (otherwise idle in phase A) gpsimd queue, where their
            # wait for an xa slot blocks nothing.
            t = _consts(nc, tc, top, d, loads=False)
            with ExitStack() as pa:
                pA = _phaseA_pools(tc, pa)
                xts_pre = [_load_x_chunk(nc, d, pA, 0)]
                _consts_load_down(nc, t, d)
                xts_pre.append(_load_x_chunk(nc, d, pA, 1))
                _consts_load_rest(nc, t, d)
                xts_pre.append(_load_x_chunk(nc, d, pA, 2, eng=nc.gpsimd))
                xts_pre.append(_load_x_chunk(nc, d, pA, 3, eng=nc.gpsimd))
                _emit_A(nc, tc, d, t, pA, xts_pre)
            _load_wo(nc, t, d)  # hide the Wo load under phase B
            with ExitStack() as pb:
                att_pool = pb.enter_context(tc.tile_pool(name="att", bufs=1))
                att_t = [att_pool.tile([128, S], BF16, name=f"att{h}")
                         for h in range(HPC)]
                with ExitStack() as pbi:
                    pB = _phaseB_pools(tc, pbi)
                    _emit_B(nc, tc, d, t, pB, att_t)
                with ExitStack() as pc:
                    pC = _phaseC_pools(tc, pc)
                    _emit_C(nc, tc, d, t, pC, att_t)
        else:
            phase, n = loop
            t = _consts(nc, tc, top, d)

            def _fill(tile_, w):
                nc.sync.dma_start(tile_[:], d["xT"][0:128, 0:w])

            with ExitStack() as ps_:
                if phase == "A":
                    pA = _phaseA_pools(tc, ps_)
                    with tc.For_i(0, n, 1):
                        _emit_A(nc, tc, d, t, pA)
                elif phase.startswith("B"):
                    mode = {"B": "full", "B0": "qk", "B1": "qke"}[phase]
                    for h in range(HPC):
                        _fill(t["K"][h], S)
                        _fill(t["Q"][h], S)
                    for i in range(NKC):
                        _fill(t["V"][i], HPC * D)
                    att_pool = ps_.enter_context(
                        tc.tile_pool(name="att", bufs=1))
                    att_t = [att_pool.tile([128, S], BF16, name=f"att{h}")
                             for h in range(HPC)]
                    pB = _phaseB_pools(tc, ps_)
                    with tc.For_i(0, n, 1):
                        _emit_B(nc, tc, d, t, pB, att_t, mode)
                elif phase == "C":
                    att_pool = ps_.enter_context(
                        tc.tile_pool(name="att", bufs=1))
                    att_t = [att_pool.tile([128, S], BF16, name=f"att{h}")
                             for h in range(HPC)]
                    for h in range(HPC):
                        _fill(att_t[h], S)
                    _load_wo(nc, t, d)
                    pC = _phaseC_pools(tc, ps_)
                    with tc.For_i(0, n, 1):
                        _emit_C(nc, tc, d, t, pC, att_t)
                else:
                    raise ValueError(phase)

    nc.compile()
    return nc


def _bf16(a):
    import ml_dtypes
    return np.ascontiguousarray(np.asarray(a, dtype=np.float32)).astype(
        ml_dtypes.bfloat16)


def _rope_tables():
    inv_freq = 1.0 / (10000.0 ** (np.arange(0, R, 2, dtype=np.float64) / R))
    t = np.arange(S, dtype=np.float64)
    freqs = np.outer(t, inv_freq)                       # (S, R/2)
    emb = np.concatenate([freqs, freqs], axis=-1)       # (S, R)
    cos = np.cos(emb).astype(np.float32)                # (S, R)
    sin = np.sin(emb).astype(np.float32)
    perm = np.array([(j // 2) if j % 2 == 0 else (j // 2) + R // 2
                     for j in range(R)])
    sign = np.array([-1.0 if j % 2 == 0 else 1.0
                     for j in range(R)], dtype=np.float32)
    cos_p = cos[:, perm].T.copy()                       # (R, S)
    sin_p = (sin[:, perm] * sign[None, :]).T.copy()     # (R, S)
    cosT = np.concatenate([cos_p, cos_p], axis=0)       # (128, S)
    sinT = np.concatenate([sin_p, sin_p], axis=0)
    return cosT, sinT, perm


def _per_core_inputs(inputs, core):
    b, hg = divmod(core, HPC)
    cosT, sinT, perm = _rope_tables()
    hsl64 = np.concatenate([hg * HPC * R + h * R + perm
                            for h in range(HPC)])       # permuted rope rows
    hs64 = slice(hg * HPC * R, (hg + 1) * HPC * R)      # natural 64-rows
    hs128 = slice(hg * HPC * D, (hg + 1) * HPC * D)     # natural 128-rows

    x = np.asarray(inputs["x"], dtype=np.float32)
    f = np.float32
    im = {
        "xT": _bf16(x[b].T),
        "wkvdT": _bf16(np.asarray(inputs["Wkv_d"], f).T),
        "wqdT": _bf16(np.asarray(inputs["Wq_d"], f).T),
        "wrkT": _bf16(np.asarray(inputs["Wrk"], f)[hsl64].T),
        "wkuT": _bf16(np.asarray(inputs["Wk_u"], f)[hs64].T),
        "wquT": _bf16(np.asarray(inputs["Wq_u"], f)[hs64].T),
        "wrqT": _bf16(np.asarray(inputs["Wrq"], f)[hsl64].T),
        "wvuT": _bf16(np.asarray(inputs["Wv_u"], f)[hs128].T),
        "woT": _bf16(np.asarray(inputs["Wo"], f).T[hs128]),
        "bkvd": np.ascontiguousarray(
            np.asarray(inputs["bkv_d"], f).reshape(NL, 128).T),
        "bqd": np.ascontiguousarray(
            np.asarray(inputs["bq_d"], f).reshape(NL, 128).T),
        "bku": np.ascontiguousarray(
            np.asarray(inputs["bk_u"], f)[hs64].reshape(2, 128).T),
        "bqu": np.ascontiguousarray(
            np.asarray(inputs["bq_u"], f)[hs64].reshape(2, 128).T),
        "brk": np.ascontiguousarray(
            np.asarray(inputs["brk"], f)[hsl64].reshape(2, 128).T),
        "brq": np.ascontiguousarray(
            np.asarray(inputs["brq"], f)[hsl64].reshape(2, 128).T),
        "bvu": np.ascontiguousarray(
            np.asarray(inputs["bv_u"], f)[hs128].reshape(1, HPC * D)),
        "onesd": np.ones((128, 1), dtype=np.float32),
        "cosT": _bf16(cosT),
        "sinT": _bf16(sinT),
    }
    return im


def _get_runtime(loop=None):
    key = loop
    if key in _RT:
        return _RT[key]
    import jax
    import numpy as _np
    from jax.sharding import Mesh, PartitionSpec
    from jax.experimental.shard_map import shard_map

    import concourse.mybir as mybir
    from concourse import bass2jax

    nc = _build_program(loop)
    bass2jax.install_neuronx_cc_hook()

    partition_name = (nc.partition_id_tensor.name
                      if nc.partition_id_tensor else None)
    in_names, out_names, out_avals, zero_shapes = [], [], [], []
    for alloc in nc.m.functions[0].allocations:
        if not isinstance(alloc, mybir.MemoryLocationSet):
            continue
        name = alloc.memorylocations[0].name
        if alloc.kind == "ExternalInput":
            if name != partition_name:
                in_names.append(name)
        elif alloc.kind == "ExternalOutput":
            out_names.append(name)
            np_dt = mybir.dt.np(alloc.dtype)
            out_avals.append(jax.core.ShapedArray(
                tuple(alloc.tensor_shape), np_dt))
            zero_shapes.append((tuple(alloc.tensor_shape), np_dt))

    n_params = len(in_names)
    n_outs = len(out_names)
    all_in_names = list(in_names) + list(out_names)
    if partition_name is not None:
        all_in_names.append(partition_name)

    def _body(*args):
        operands = list(args)
        if partition_name is not None:
            operands.append(bass2jax.partition_id_tensor())
        outs = bass2jax._bass_exec_p.bind(
            *operands,
            out_avals=tuple(out_avals),
            in_names=tuple(all_in_names),
            out_names=tuple(out_names),
            lowering_input_output_aliases=(),
            sim_require_finite=True,
            sim_require_nnan=True,
            nc=nc,
        )
        return tuple(outs)

    devices = jax.devices()[:NCORES]
    mesh = Mesh(_np.asarray(devices), ("core",))
    in_specs = (PartitionSpec("core"),) * (n_params + n_outs)
    out_specs = (PartitionSpec("core"),) * n_outs
    donate = tuple(range(n_params, n_params + n_outs))
    sharded = jax.jit(
        shard_map(_body, mesh=mesh, in_specs=in_specs, out_specs=out_specs,
                  check_rep=False),
        donate_argnums=donate, keep_unused=True)

    _RT[key] = dict(sharded=sharded, in_names=in_names, out_names=out_names,
                    zero_shapes=zero_shapes, n_outs=n_outs, nc=nc)
    return _RT[key]


def _run_cores(in_maps):
    rt = _get_runtime()
    import numpy as _np
    concat_in = [
        _np.concatenate([in_maps[c][name] for c in range(NCORES)], axis=0)
        for name in rt["in_names"]
    ]
    concat_zeros = [
        _np.zeros((NCORES * shp[0],) + shp[1:], dt)
        for (shp, dt) in rt["zero_shapes"]
    ]
    out_arrs = rt["sharded"](*concat_in, *concat_zeros)
    res = []
    for c in range(NCORES):
        m = {}
        for i, name in enumerate(rt["out_names"]):
            shp, dt = rt["zero_shapes"][i]
            m[name] = _np.asarray(out_arrs[i]).reshape((NCORES,) + shp)[c]
        res.append(m)
    return res


def kernel(**inputs):
    in_maps = [_per_core_inputs(inputs, c) for c in range(NCORES)]
    res = _run_cores(in_maps)
    bo = np.asarray(inputs["bo"], dtype=np.float32)
    final = np.empty((B, S, E), dtype=np.float32)
    for b in range(B):
        acc = res[HPC * b]["out"].astype(np.float32).copy()
        for g in range(1, HPC):
            acc += res[HPC * b + g]["out"]
        final[b] = acc + bo[None, :]
    return final
